# revision 58
# baseline (speedup 1.0000x reference)
"""Multi-head attention (S=2048, B=2, D=1024, H=16) on 8 Trainium2 NeuronCores.

Sharding: batch x head-group. Core c handles batch c//4 and heads
[4*(c%4), 4*(c%4)+4). Each core computes its 4 heads' Q/K/V projections,
attention, and a partial output projection (row-parallel Wo); the host sums
the 4 partials per batch and adds the bias terms (bo and the exact wo@bv
correction; softmax rows sum to 1 so bv folds out of the attention).

Device-side structure (per core):
  - inputs pre-transposed on host: xq_t/xk_t/xv_t (D, S) so the projection
    contraction dim (d) lands on SBUF partitions.
  - QT/KT (dk-major, 2 tiles of (128, S)): one head pair per tile, f32r.
  - scores computed transposed, ST = (j, i), via fp32r matmuls; the two heads
    of a pair run concurrently in disjoint PE row groups (K=dk=64 each).
  - softmax: exp on ScalarE straight out of PSUM with the 1/sqrt(dk) scale
    folded into the activation; no max subtraction (scores are O(1) here);
    normalization is deferred past P@V by appending a ones column to V so the
    PE accumulates the denominator Z next to U = exp(S^T).T @ V.
  - O = U * (1/Z) per query row (per-partition scalars on DVE), PE-transposed,
    then the output projection runs in f32r.
  - projections are emitted column-block-wise through a worklist drained
    inside the attention J-loop so DMA/PE/ACT/DVE overlap end to end.
"""

import sys

sys.path.insert(0, "/opt/trn_rl_repo")

from collections import deque

import numpy as np

import concourse.bass as bass
import concourse.tile as tile
from concourse import bacc, mybir
from concourse.bass_utils import run_bass_kernel_spmd
from concourse.masks import make_identity

S = 2048
B = 2
D = 1024
H = 16
DK = 64
G = 4            # heads per core
DC = G * DK      # 256 per-core head dims
SCALE = 1.0 / np.sqrt(DK)  # 0.125
P = 128
NJ = S // P      # 16 key chunks
NIT = S // P     # 16 query tiles
NIB = 4          # i blocks of 512
IB = S // NIB    # 512
ND = D // P      # 8 contraction chunks for projections

F32 = mybir.dt.float32
F32R = mybir.dt.float32r
BF16 = mybir.dt.bfloat16
FP16 = mybir.dt.float16
EXP = mybir.ActivationFunctionType.Exp
ADD = mybir.AluOpType.add
MULT = mybir.AluOpType.mult

_NC_CACHE = None


def _build(dbg=False, st_dtype=F32R):
    nc = bacc.Bacc("TRN2", target_bir_lowering=False, debug=False)

    xq_t = nc.dram_tensor("xq_t", [D, S], F32R, kind="ExternalInput")
    xk_t = nc.dram_tensor("xk_t", [D, S], F32R, kind="ExternalInput")
    xv_t = nc.dram_tensor("xv_t", [D, S], F32R, kind="ExternalInput")
    wq_t = nc.dram_tensor("wq_t", [D, DC], F32R, kind="ExternalInput")
    wk_t = nc.dram_tensor("wk_t", [D, DC], F32R, kind="ExternalInput")
    wv_t = nc.dram_tensor("wv_t", [D, DC], F32R, kind="ExternalInput")
    wo_t = nc.dram_tensor("wo_t", [DC, D], F32R, kind="ExternalInput")
    bq_s = nc.dram_tensor("bq_s", [P, 2], F32, kind="ExternalInput")
    bk_s = nc.dram_tensor("bk_s", [P, 2], F32, kind="ExternalInput")
    y = nc.dram_tensor("y", [S, D], F32, kind="ExternalOutput")
    if dbg:
        qk_dt = F32 if st_dtype == F32R else st_dtype
        d_qt = nc.dram_tensor("d_qt", [2, P, S], qk_dt, kind="ExternalOutput")
        d_kt = nc.dram_tensor("d_kt", [2, P, S], qk_dt, kind="ExternalOutput")
        d_va = nc.dram_tensor("d_va", [P, NJ, G * 65], FP16, kind="ExternalOutput")
        d_o = nc.dram_tensor("d_o", [P, NIT, DC], F32, kind="ExternalOutput")
        d_et = nc.dram_tensor("d_et", [P, 2 * IB], FP16, kind="ExternalOutput")
        d_u = nc.dram_tensor("d_u", [2, P, 4 * 65], F32, kind="ExternalOutput")

    with tile.TileContext(nc) as tc:
        with (
            tc.tile_pool(name="persist", bufs=1) as persist,
            tc.tile_pool(name="xs", bufs=24) as xs,
            tc.tile_pool(name="ps", bufs=2, space="PSUM") as psp,   # st/proj shared
            tc.tile_pool(name="up", bufs=4, space="PSUM") as up,    # U pairs + phase C
            tc.tile_pool(name="et", bufs=6) as etp,
            tc.tile_pool(name="rz", bufs=2) as rzp,
            tc.tile_pool(name="ysb", bufs=2) as ysb,
        ):
            # ---- persistent SBUF (DMAs ordered by first use) ----
            wq_sb = persist.tile([P, ND, DC], F32R)
            wk_sb = persist.tile([P, ND, DC], F32R)
            wv_sb = persist.tile([P, ND, DC], F32R)
            bq_sb = persist.tile([P, 2], F32)
            bk_sb = persist.tile([P, 2], F32)
            nc.sync.dma_start(out=wk_sb, in_=wk_t.ap().rearrange("(c p) m -> p c m", p=P))
            nc.sync.dma_start(out=bk_sb, in_=bk_s.ap())
            nc.sync.dma_start(out=bq_sb, in_=bq_s.ap())
            woc_sb = persist.tile([P, 2, D], F32R)

            qt_sb = [persist.tile([P, S], st_dtype, tag=f"qt{t}", name=f"qt{t}") for t in range(2)]
            kt_sb = [persist.tile([P, S], st_dtype, tag=f"kt{t}", name=f"kt{t}") for t in range(2)]
            vaug = persist.tile([P, NJ, G * 65], FP16)
            for h in range(G):
                nc.vector.memset(vaug[:, :, h * 65 + 64 : h * 65 + 65], 1.0)
            o_sb = persist.tile([P, NIT, DC], F32R)
            ot_sb = [persist.tile([P, S], F32R, tag=f"ot{t}", name=f"ot{t}") for t in range(2)]
            ident_f = persist.tile([P, P], F32)
            make_identity(nc, ident_f)
            ident = persist.tile([P, P], F32R)
            nc.vector.tensor_copy(ident, ident_f)

            # ---- work items: column-block DMA + projections, phase-C steps ----
            slabs = {}       # (tensor_key, cb) -> list of 8 (128, IB) tiles
            emitted = set()  # work-item keys already emitted

            def dma_slab(key, xt, cb):
                tiles = []
                for dc in range(ND):
                    t = xs.tile([P, IB], F32R, tag="x", name="x")
                    nc.sync.dma_start(
                        out=t, in_=xt.ap()[dc * P : (dc + 1) * P, cb * IB : (cb + 1) * IB]
                    )
                    tiles.append(t)
                slabs[(key, cb)] = tiles
                emitted.add(("dma", key, cb))

            def proj_qk(key, cb, mt, half, w_sb, b_sb, out_tiles):
                # half-width (256-col) groups keep the PSUM slot held for
                # less than one exp tick, so background projection work
                # never stalls the attention pipeline.
                HW_ = IB // 2
                ps = psp.tile([P, HW_], F32, tag="ps", name="ps")
                x_tiles = slabs[(key, cb)]
                for dc in range(ND):
                    nc.tensor.matmul(
                        ps,
                        w_sb[:, dc, mt * P : (mt + 1) * P],
                        x_tiles[dc][:, half * HW_ : (half + 1) * HW_],
                        start=(dc == 0),
                        stop=(dc == ND - 1),
                    )
                nc.vector.tensor_scalar(
                    out_tiles[mt][:, cb * IB + half * HW_ : cb * IB + (half + 1) * HW_],
                    ps,
                    b_sb[:, mt : mt + 1],
                    None,
                    op0=ADD,
                )
                emitted.add((key, cb, mt, half))

            def proj_v(cb, jq):
                # jq: 0..3 within the column block; global j chunk jt
                jt = cb * 4 + jq
                ps = psp.tile([P, DC], F32, tag="ps", name="ps")
                x_tiles = slabs[("v", cb)]
                for dc in range(ND):
                    nc.tensor.matmul(
                        ps,
                        x_tiles[dc][:, jq * P : (jq + 1) * P],
                        wv_sb[:, dc, :],
                        start=(dc == 0),
                        stop=(dc == ND - 1),
                    )
                nc.vector.tensor_copy(
                    vaug[:, jt].rearrange("p (h c) -> p h c", h=G)[:, :, 0:DK],
                    ps.rearrange("p (h c) -> p h c", h=G),
                )
                emitted.add(("v", cb, jq))

            def oproj(it):
                ys = ysb.tile([P, D], F32, tag="ysb", name="ysb")
                for nh in range(2):
                    yp = up.tile([P, IB], F32, tag="u", name="yp")
                    for mt in range(2):
                        nc.tensor.matmul(
                            yp,
                            ot_sb[mt][:, it * P : (it + 1) * P],
                            woc_sb[:, mt, nh * IB : (nh + 1) * IB],
                            start=(mt == 0),
                            stop=(mt == 1),
                        )
                    nc.vector.tensor_copy(ys[:, nh * IB : (nh + 1) * IB], yp)
                nc.sync.dma_start(out=y.ap()[it * P : (it + 1) * P, :], in_=ys)

            def phase_c(ib):
                items = [
                    lambda mt=mt, pair=pair: transp2(ib, mt, pair)
                    for mt in range(2)
                    for pair in range(2)
                ]
                items += [lambda it=it: oproj(it) for it in range(ib * 4, ib * 4 + 4)]
                return items

            def phase_c_late(ib):
                # hp0 transposes were already queued during the last sweep
                items = [lambda pair=pair: transp2(ib, 1, pair) for pair in range(2)]
                items += [lambda it=it: oproj(it) for it in range(ib * 4, ib * 4 + 4)]
                return items

            def transp2(ib, mt, pair):
                for it in range(ib * 4 + 2 * pair, ib * 4 + 2 * pair + 2):
                    tp = up.tile([P, P], F32R, tag="u", name="tp")
                    nc.tensor.transpose(
                        tp, o_sb[:, it, mt * P : (mt + 1) * P], ident
                    )
                    nc.vector.tensor_copy(
                        ot_sb[mt][:, it * P : (it + 1) * P], tp
                    )

            work = deque()    # psum-holding items: paced to alternate ticks
            light = deque()   # DMA/staging items: drained every tick

            def drain(tick):
                if light:
                    light.popleft()()
                if work and (tick % 2 == 0 or not light):
                    work.popleft()()

            def drain_until(key):
                while key not in emitted:
                    assert work or light, f"work exhausted before {key}"
                    if light:
                        light.popleft()()
                    elif work:
                        work.popleft()()

            # column block 0 emitted up front (the pipeline fill); cb0's
            # V-projection drains inside the first J sweep instead of
            # stalling the PE stream ahead of the first score matmul.
            dma_slab("k", xk_t, 0)
            for half in range(2):
                proj_qk("k", 0, 0, half, wk_sb, bk_sb, kt_sb)
                proj_qk("k", 0, 1, half, wk_sb, bk_sb, kt_sb)
            nc.sync.dma_start(out=wq_sb, in_=wq_t.ap().rearrange("(c p) m -> p c m", p=P))
            dma_slab("q", xq_t, 0)
            for half in range(2):
                proj_qk("q", 0, 0, half, wq_sb, bq_sb, qt_sb)
                proj_qk("q", 0, 1, half, wq_sb, bq_sb, qt_sb)
            nc.sync.dma_start(out=wv_sb, in_=wv_t.ap().rearrange("(c p) m -> p c m", p=P))
            dma_slab("v", xv_t, 0)

            def load_woc():
                nc.sync.dma_start(
                    out=woc_sb, in_=wo_t.ap().rearrange("(t p) n -> p t n", p=P)
                )

            for jq in range(4):
                work.append(lambda jq=jq: proj_v(0, jq))
            for cb in range(1, NIB):
                light.append(lambda cb=cb: dma_slab("k", xk_t, cb))
                for half in range(2):
                    work.append(lambda cb=cb, half=half: proj_qk("k", cb, 0, half, wk_sb, bk_sb, kt_sb))
                    work.append(lambda cb=cb, half=half: proj_qk("k", cb, 1, half, wk_sb, bk_sb, kt_sb))
                light.append(lambda cb=cb: dma_slab("v", xv_t, cb))
                for jq in range(4):
                    work.append(lambda cb=cb, jq=jq: proj_v(cb, jq))
            light.append(load_woc)
            for cb in range(1, NIB):
                light.append(lambda cb=cb: dma_slab("q", xq_t, cb))
                for half in range(2):
                    work.append(lambda cb=cb, half=half: proj_qk("q", cb, 0, half, wq_sb, bq_sb, qt_sb))
                    work.append(lambda cb=cb, half=half: proj_qk("q", cb, 1, half, wq_sb, bq_sb, qt_sb))

            # ---- attention: flat software pipeline over (ib, hp, J) ticks ----
            # scores+exp for tick t are emitted one tick ahead of tick t-1's
            # P@V matmuls, so the exp stream never waits on a pair boundary.
            seq = [(0, hp, jblk * 4 + J) for jblk in range(4) for hp in range(2) for J in range(4)]
            seq += [(ib, hp, J) for ib in range(1, NIB) for hp in range(2) for J in range(NJ)]
            u_tiles = {}   # (ib, hp) -> [uA, uB]
            et_tiles = {}  # tick index -> et tile

            def emit_st_exp(idx):
                ib, hp, J = seq[idx]
                if hp == 0 and J == 0:
                    for mt in range(2):
                        for half in range(2):
                            drain_until(("q", ib, mt, half))
                if ib == 0 and hp == 0:
                    for mt in range(2):
                        for half in range(2):
                            drain_until(("k", J // 4, mt, half))
                st = psp.tile([P, 2 * IB], F32, tag="ps", name="st")
                for hx in range(2):
                    nc.tensor.matmul(
                        st[:, hx * IB : (hx + 1) * IB],
                        kt_sb[hp][hx * DK : (hx + 1) * DK, J * P : (J + 1) * P],
                        qt_sb[hp][hx * DK : (hx + 1) * DK, ib * IB : (ib + 1) * IB],
                        start=True,
                        stop=True,
                        tile_position=(hx * DK, 0),
                    )
                et = etp.tile([P, 2 * IB], FP16, tag="et", name="et")
                nc.scalar.activation(et, st, EXP, scale=float(SCALE))
                et_tiles[idx] = et
                if dbg and idx == 0:
                    nc.sync.dma_start(out=d_et.ap(), in_=et)

            def emit_pv(idx):
                ib, hp, J = seq[idx]
                if J == 0:
                    u_tiles[(ib, hp)] = [
                        up.tile([P, 4 * 65], F32, tag="u", name="u") for _ in range(2)
                    ]
                if ib == 0 and hp == 0:
                    drain_until(("v", J // 4, J % 4))
                et = et_tiles.pop(idx)
                u_hx = u_tiles[(ib, hp)]
                for hx in range(2):
                    h = 2 * hp + hx
                    for it in range(4):
                        # start clears has_written for the WHOLE bank: only the
                        # first matmul into this U bank may set it.
                        nc.tensor.matmul(
                            u_hx[hx][:, it * 65 : (it + 1) * 65],
                            et[:, hx * IB + it * P : hx * IB + (it + 1) * P],
                            vaug[:, J, h * 65 : (h + 1) * 65],
                            start=(J == 0 and it == 0),
                            stop=(J == NJ - 1 and it == 3),
                            skip_group_check=True,
                        )
                if J == NJ - 1:
                    finish_pair(ib, hp)

            def finish_pair(ib, hp):
                u_hx = u_tiles.pop((ib, hp))
                if dbg and ib == 0 and hp == 0:
                    for hx in range(2):
                        du_sb = ysb.tile([P, 4 * 65], F32, tag="du", name="du")
                        nc.vector.tensor_copy(du_sb, u_hx[hx])
                        nc.sync.dma_start(out=d_u.ap()[hx], in_=du_sb)
                for hx in range(2):
                    h = 2 * hp + hx
                    uv = u_hx[hx].rearrange("p (r c) -> p r c", c=65)
                    rz = rzp.tile([P, 4, 1], F32, tag="rz", name="rz")
                    nc.vector.reciprocal(rz, uv[:, :, 64:65])
                    for it in range(4):
                        nc.vector.tensor_scalar(
                            o_sb[:, ib * 4 + it, h * DK : (h + 1) * DK],
                            uv[:, it, 0:DK],
                            rz[:, it],
                            None,
                            op0=MULT,
                        )
                if hp == 0 and ib == NIB - 1:
                    # last i-block: transpose the hp0 heads during the final
                    # sweep so less phase-C work serializes after the last exp
                    work.extend(
                        [lambda pair=pair: transp2(NIB - 1, 0, pair) for pair in range(2)]
                    )
                elif hp == 1:
                    items = phase_c(ib) if ib < NIB - 1 else phase_c_late(ib)
                    work.extend(items)

            for idx in range(len(seq) + 1):
                if idx < len(seq):
                    emit_st_exp(idx)
                if idx >= 1:
                    emit_pv(idx - 1)
                drain(idx)

            while work or light:
                (light or work).popleft()()

            if dbg:
                for t in range(2):
                    qsrc = qt_sb[t].bitcast(qk_dt) if st_dtype == F32R else qt_sb[t]
                    ksrc = kt_sb[t].bitcast(qk_dt) if st_dtype == F32R else kt_sb[t]
                    nc.sync.dma_start(out=d_qt.ap()[t], in_=qsrc)
                    nc.sync.dma_start(out=d_kt.ap()[t], in_=ksrc)
                nc.sync.dma_start(out=d_va.ap(), in_=vaug)
                nc.sync.dma_start(out=d_o.ap(), in_=o_sb.bitcast(F32))

    nc.compile()
    return nc


def _get_nc():
    global _NC_CACHE
    if _NC_CACHE is None:
        _NC_CACHE = _build()
    return _NC_CACHE


def _in_maps(query, key, value, wq, wk, wv, wo, bq, bk):
    maps = []
    for c in range(8):
        b, g = divmod(c, 4)
        sl = slice(g * DC, (g + 1) * DC)
        maps.append(
            {
                "xq_t": np.ascontiguousarray(query[:, b, :].T),
                "xk_t": np.ascontiguousarray(key[:, b, :].T),
                "xv_t": np.ascontiguousarray(value[:, b, :].T),
                "wq_t": np.ascontiguousarray(wq[sl, :].T),
                "wk_t": np.ascontiguousarray(wk[sl, :].T),
                "wv_t": np.ascontiguousarray(wv[sl, :].T),
                "wo_t": np.ascontiguousarray(wo[:, sl].T),
                "bq_s": np.ascontiguousarray(bq[sl].reshape(2, P).T),
                "bk_s": np.ascontiguousarray(bk[sl].reshape(2, P).T),
            }
        )
    return maps


def kernel(
    query, key, value, wq, bq, wk, bk, wv, bv, wo, bo, **_kw
) -> np.ndarray:
    query = np.asarray(query, np.float32)
    key = np.asarray(key, np.float32)
    value = np.asarray(value, np.float32)
    wq = np.asarray(wq, np.float32)
    wk = np.asarray(wk, np.float32)
    wv = np.asarray(wv, np.float32)
    wo = np.asarray(wo, np.float32)
    bq = np.asarray(bq, np.float32)
    bk = np.asarray(bk, np.float32)
    bv = np.asarray(bv, np.float32)
    bo = np.asarray(bo, np.float32)

    nc = _get_nc()
    res = run_bass_kernel_spmd(
        nc, _in_maps(query, key, value, wq, wk, wv, wo, bq, bk),
        core_ids=list(range(8)),
    )

    out = np.zeros((S, B, D), np.float32)
    for c in range(8):
        out[:, c // 4, :] += res.results[c]["y"]
    out += bo + wo @ bv
    return out


# revision 59
# speedup vs baseline: 1.0007x; 1.0007x over previous
"""Multi-head attention (S=2048, B=2, D=1024, H=16) on 8 Trainium2 NeuronCores.

Sharding: batch x head-group. Core c handles batch c//4 and heads
[4*(c%4), 4*(c%4)+4). Each core computes its 4 heads' Q/K/V projections,
attention, and a partial output projection (row-parallel Wo); the host sums
the 4 partials per batch and adds the bias terms (bo and the exact wo@bv
correction; softmax rows sum to 1 so bv folds out of the attention).

Device-side structure (per core):
  - inputs pre-transposed on host: xq_t/xk_t/xv_t (D, S) so the projection
    contraction dim (d) lands on SBUF partitions.
  - QT/KT (dk-major, 2 tiles of (128, S)): one head pair per tile, f32r.
  - scores computed transposed, ST = (j, i), via fp32r matmuls; the two heads
    of a pair run concurrently in disjoint PE row groups (K=dk=64 each).
  - softmax: exp on ScalarE straight out of PSUM with the 1/sqrt(dk) scale
    folded into the activation; no max subtraction (scores are O(1) here);
    normalization is deferred past P@V by appending a ones column to V so the
    PE accumulates the denominator Z next to U = exp(S^T).T @ V.
  - O = U * (1/Z) per query row (per-partition scalars on DVE), PE-transposed,
    then the output projection runs in f32r.
  - projections are emitted column-block-wise through a worklist drained
    inside the attention J-loop so DMA/PE/ACT/DVE overlap end to end.
"""

import sys

sys.path.insert(0, "/opt/trn_rl_repo")

from collections import deque

import numpy as np

import concourse.bass as bass
import concourse.tile as tile
from concourse import bacc, mybir
from concourse.bass_utils import run_bass_kernel_spmd
from concourse.masks import make_identity

S = 2048
B = 2
D = 1024
H = 16
DK = 64
G = 4            # heads per core
DC = G * DK      # 256 per-core head dims
SCALE = 1.0 / np.sqrt(DK)  # 0.125
P = 128
NJ = S // P      # 16 key chunks
NIT = S // P     # 16 query tiles
NIB = 4          # i blocks of 512
IB = S // NIB    # 512
ND = D // P      # 8 contraction chunks for projections

F32 = mybir.dt.float32
F32R = mybir.dt.float32r
BF16 = mybir.dt.bfloat16
FP16 = mybir.dt.float16
EXP = mybir.ActivationFunctionType.Exp
ADD = mybir.AluOpType.add
MULT = mybir.AluOpType.mult

_NC_CACHE = None


def _build(dbg=False, st_dtype=F32R):
    nc = bacc.Bacc("TRN2", target_bir_lowering=False, debug=False)

    xq_t = nc.dram_tensor("xq_t", [D, S], F32R, kind="ExternalInput")
    xk_t = nc.dram_tensor("xk_t", [D, S], F32R, kind="ExternalInput")
    xv_t = nc.dram_tensor("xv_t", [D, S], F32R, kind="ExternalInput")
    wq_t = nc.dram_tensor("wq_t", [D, DC], F32R, kind="ExternalInput")
    wk_t = nc.dram_tensor("wk_t", [D, DC], F32R, kind="ExternalInput")
    wv_t = nc.dram_tensor("wv_t", [D, DC], F32R, kind="ExternalInput")
    wo_t = nc.dram_tensor("wo_t", [DC, D], F32R, kind="ExternalInput")
    bq_s = nc.dram_tensor("bq_s", [P, 2], F32, kind="ExternalInput")
    bk_s = nc.dram_tensor("bk_s", [P, 2], F32, kind="ExternalInput")
    y = nc.dram_tensor("y", [S, D], F32, kind="ExternalOutput")
    if dbg:
        qk_dt = F32 if st_dtype == F32R else st_dtype
        d_qt = nc.dram_tensor("d_qt", [2, P, S], qk_dt, kind="ExternalOutput")
        d_kt = nc.dram_tensor("d_kt", [2, P, S], qk_dt, kind="ExternalOutput")
        d_va = nc.dram_tensor("d_va", [P, NJ, G * 65], FP16, kind="ExternalOutput")
        d_o = nc.dram_tensor("d_o", [P, NIT, DC], F32, kind="ExternalOutput")
        d_et = nc.dram_tensor("d_et", [P, 2 * IB], FP16, kind="ExternalOutput")
        d_u = nc.dram_tensor("d_u", [2, P, 4 * 65], F32, kind="ExternalOutput")

    with tile.TileContext(nc) as tc:
        with (
            tc.tile_pool(name="persist", bufs=1) as persist,
            tc.tile_pool(name="xs", bufs=24) as xs,
            tc.tile_pool(name="ps", bufs=2, space="PSUM") as psp,   # st/proj shared
            tc.tile_pool(name="up", bufs=4, space="PSUM") as up,    # U pairs + phase C
            tc.tile_pool(name="et", bufs=8) as etp,
            tc.tile_pool(name="rz", bufs=2) as rzp,
            tc.tile_pool(name="ysb", bufs=2) as ysb,
        ):
            # ---- persistent SBUF (DMAs ordered by first use) ----
            wq_sb = persist.tile([P, ND, DC], F32R)
            wk_sb = persist.tile([P, ND, DC], F32R)
            wv_sb = persist.tile([P, ND, DC], F32R)
            bq_sb = persist.tile([P, 2], F32)
            bk_sb = persist.tile([P, 2], F32)
            nc.sync.dma_start(out=wk_sb, in_=wk_t.ap().rearrange("(c p) m -> p c m", p=P))
            nc.sync.dma_start(out=bk_sb, in_=bk_s.ap())
            nc.sync.dma_start(out=bq_sb, in_=bq_s.ap())
            woc_sb = persist.tile([P, 2, D], F32R)

            qt_sb = [persist.tile([P, S], st_dtype, tag=f"qt{t}", name=f"qt{t}") for t in range(2)]
            kt_sb = [persist.tile([P, S], st_dtype, tag=f"kt{t}", name=f"kt{t}") for t in range(2)]
            vaug = persist.tile([P, NJ, G * 65], FP16)
            for h in range(G):
                nc.vector.memset(vaug[:, :, h * 65 + 64 : h * 65 + 65], 1.0)
            o_sb = persist.tile([P, NIT, DC], F32R)
            ot_sb = [persist.tile([P, S], F32R, tag=f"ot{t}", name=f"ot{t}") for t in range(2)]
            ident_f = persist.tile([P, P], F32)
            make_identity(nc, ident_f)
            ident = persist.tile([P, P], F32R)
            nc.vector.tensor_copy(ident, ident_f)

            # ---- work items: column-block DMA + projections, phase-C steps ----
            slabs = {}       # (tensor_key, cb) -> list of 8 (128, IB) tiles
            emitted = set()  # work-item keys already emitted

            def dma_slab(key, xt, cb):
                tiles = []
                for dc in range(ND):
                    t = xs.tile([P, IB], F32R, tag="x", name="x")
                    nc.sync.dma_start(
                        out=t, in_=xt.ap()[dc * P : (dc + 1) * P, cb * IB : (cb + 1) * IB]
                    )
                    tiles.append(t)
                slabs[(key, cb)] = tiles
                emitted.add(("dma", key, cb))

            def proj_qk(key, cb, mt, half, w_sb, b_sb, out_tiles):
                # half-width (256-col) groups keep the PSUM slot held for
                # less than one exp tick, so background projection work
                # never stalls the attention pipeline.
                HW_ = IB // 2
                ps = psp.tile([P, HW_], F32, tag="ps", name="ps")
                x_tiles = slabs[(key, cb)]
                for dc in range(ND):
                    nc.tensor.matmul(
                        ps,
                        w_sb[:, dc, mt * P : (mt + 1) * P],
                        x_tiles[dc][:, half * HW_ : (half + 1) * HW_],
                        start=(dc == 0),
                        stop=(dc == ND - 1),
                    )
                nc.vector.tensor_scalar(
                    out_tiles[mt][:, cb * IB + half * HW_ : cb * IB + (half + 1) * HW_],
                    ps,
                    b_sb[:, mt : mt + 1],
                    None,
                    op0=ADD,
                )
                emitted.add((key, cb, mt, half))

            def proj_v(cb, jq):
                # jq: 0..3 within the column block; global j chunk jt
                jt = cb * 4 + jq
                ps = psp.tile([P, DC], F32, tag="ps", name="ps")
                x_tiles = slabs[("v", cb)]
                for dc in range(ND):
                    nc.tensor.matmul(
                        ps,
                        x_tiles[dc][:, jq * P : (jq + 1) * P],
                        wv_sb[:, dc, :],
                        start=(dc == 0),
                        stop=(dc == ND - 1),
                    )
                nc.vector.tensor_copy(
                    vaug[:, jt].rearrange("p (h c) -> p h c", h=G)[:, :, 0:DK],
                    ps.rearrange("p (h c) -> p h c", h=G),
                )
                emitted.add(("v", cb, jq))

            def oproj(it):
                ys = ysb.tile([P, D], F32, tag="ysb", name="ysb")
                for nh in range(2):
                    yp = up.tile([P, IB], F32, tag="u", name="yp")
                    for mt in range(2):
                        nc.tensor.matmul(
                            yp,
                            ot_sb[mt][:, it * P : (it + 1) * P],
                            woc_sb[:, mt, nh * IB : (nh + 1) * IB],
                            start=(mt == 0),
                            stop=(mt == 1),
                        )
                    nc.vector.tensor_copy(ys[:, nh * IB : (nh + 1) * IB], yp)
                nc.sync.dma_start(out=y.ap()[it * P : (it + 1) * P, :], in_=ys)

            def phase_c(ib):
                items = [
                    lambda mt=mt, pair=pair: transp2(ib, mt, pair)
                    for mt in range(2)
                    for pair in range(2)
                ]
                items += [lambda it=it: oproj(it) for it in range(ib * 4, ib * 4 + 4)]
                return items

            def phase_c_late(ib):
                # hp0 transposes were already queued during the last sweep
                items = [lambda pair=pair: transp2(ib, 1, pair) for pair in range(2)]
                items += [lambda it=it: oproj(it) for it in range(ib * 4, ib * 4 + 4)]
                return items

            def transp2(ib, mt, pair):
                for it in range(ib * 4 + 2 * pair, ib * 4 + 2 * pair + 2):
                    tp = up.tile([P, P], F32R, tag="u", name="tp")
                    nc.tensor.transpose(
                        tp, o_sb[:, it, mt * P : (mt + 1) * P], ident
                    )
                    nc.vector.tensor_copy(
                        ot_sb[mt][:, it * P : (it + 1) * P], tp
                    )

            work = deque()    # psum-holding items: paced to alternate ticks
            light = deque()   # DMA/staging items: drained every tick

            def drain(tick):
                if light:
                    light.popleft()()
                if work and (tick % 2 == 0 or not light):
                    work.popleft()()

            def drain_until(key):
                while key not in emitted:
                    assert work or light, f"work exhausted before {key}"
                    if light:
                        light.popleft()()
                    elif work:
                        work.popleft()()

            # column block 0 emitted up front (the pipeline fill); cb0's
            # V-projection drains inside the first J sweep instead of
            # stalling the PE stream ahead of the first score matmul.
            dma_slab("k", xk_t, 0)
            for half in range(2):
                proj_qk("k", 0, 0, half, wk_sb, bk_sb, kt_sb)
                proj_qk("k", 0, 1, half, wk_sb, bk_sb, kt_sb)
            nc.sync.dma_start(out=wq_sb, in_=wq_t.ap().rearrange("(c p) m -> p c m", p=P))
            dma_slab("q", xq_t, 0)
            for half in range(2):
                proj_qk("q", 0, 0, half, wq_sb, bq_sb, qt_sb)
                proj_qk("q", 0, 1, half, wq_sb, bq_sb, qt_sb)
            nc.sync.dma_start(out=wv_sb, in_=wv_t.ap().rearrange("(c p) m -> p c m", p=P))
            dma_slab("v", xv_t, 0)

            def load_woc():
                nc.sync.dma_start(
                    out=woc_sb, in_=wo_t.ap().rearrange("(t p) n -> p t n", p=P)
                )

            for jq in range(4):
                work.append(lambda jq=jq: proj_v(0, jq))
            for cb in range(1, NIB):
                light.append(lambda cb=cb: dma_slab("k", xk_t, cb))
                for half in range(2):
                    work.append(lambda cb=cb, half=half: proj_qk("k", cb, 0, half, wk_sb, bk_sb, kt_sb))
                    work.append(lambda cb=cb, half=half: proj_qk("k", cb, 1, half, wk_sb, bk_sb, kt_sb))
                light.append(lambda cb=cb: dma_slab("v", xv_t, cb))
                for jq in range(4):
                    work.append(lambda cb=cb, jq=jq: proj_v(cb, jq))
            light.append(load_woc)
            for cb in range(1, NIB):
                light.append(lambda cb=cb: dma_slab("q", xq_t, cb))
                for half in range(2):
                    work.append(lambda cb=cb, half=half: proj_qk("q", cb, 0, half, wq_sb, bq_sb, qt_sb))
                    work.append(lambda cb=cb, half=half: proj_qk("q", cb, 1, half, wq_sb, bq_sb, qt_sb))

            # ---- attention: flat software pipeline over (ib, hp, J) ticks ----
            # scores+exp for tick t are emitted one tick ahead of tick t-1's
            # P@V matmuls, so the exp stream never waits on a pair boundary.
            seq = [(0, hp, jblk * 4 + J) for jblk in range(4) for hp in range(2) for J in range(4)]
            seq += [(ib, hp, J) for ib in range(1, NIB) for hp in range(2) for J in range(NJ)]
            u_tiles = {}   # (ib, hp) -> [uA, uB]
            et_tiles = {}  # tick index -> et tile

            def emit_st_exp(idx):
                ib, hp, J = seq[idx]
                if hp == 0 and J == 0:
                    for mt in range(2):
                        for half in range(2):
                            drain_until(("q", ib, mt, half))
                if ib == 0 and hp == 0:
                    for mt in range(2):
                        for half in range(2):
                            drain_until(("k", J // 4, mt, half))
                st = psp.tile([P, 2 * IB], F32, tag="ps", name="st")
                for hx in range(2):
                    nc.tensor.matmul(
                        st[:, hx * IB : (hx + 1) * IB],
                        kt_sb[hp][hx * DK : (hx + 1) * DK, J * P : (J + 1) * P],
                        qt_sb[hp][hx * DK : (hx + 1) * DK, ib * IB : (ib + 1) * IB],
                        start=True,
                        stop=True,
                        tile_position=(hx * DK, 0),
                    )
                et = etp.tile([P, 2 * IB], FP16, tag="et", name="et")
                nc.scalar.activation(et, st, EXP, scale=float(SCALE))
                et_tiles[idx] = et
                if dbg and idx == 0:
                    nc.sync.dma_start(out=d_et.ap(), in_=et)

            def emit_pv(idx):
                ib, hp, J = seq[idx]
                if J == 0:
                    u_tiles[(ib, hp)] = [
                        up.tile([P, 4 * 65], F32, tag="u", name="u") for _ in range(2)
                    ]
                if ib == 0 and hp == 0:
                    drain_until(("v", J // 4, J % 4))
                et = et_tiles.pop(idx)
                u_hx = u_tiles[(ib, hp)]
                for hx in range(2):
                    h = 2 * hp + hx
                    for it in range(4):
                        # start clears has_written for the WHOLE bank: only the
                        # first matmul into this U bank may set it.
                        nc.tensor.matmul(
                            u_hx[hx][:, it * 65 : (it + 1) * 65],
                            et[:, hx * IB + it * P : hx * IB + (it + 1) * P],
                            vaug[:, J, h * 65 : (h + 1) * 65],
                            start=(J == 0 and it == 0),
                            stop=(J == NJ - 1 and it == 3),
                            skip_group_check=True,
                        )
                if J == NJ - 1:
                    finish_pair(ib, hp)

            def finish_pair(ib, hp):
                u_hx = u_tiles.pop((ib, hp))
                if dbg and ib == 0 and hp == 0:
                    for hx in range(2):
                        du_sb = ysb.tile([P, 4 * 65], F32, tag="du", name="du")
                        nc.vector.tensor_copy(du_sb, u_hx[hx])
                        nc.sync.dma_start(out=d_u.ap()[hx], in_=du_sb)
                for hx in range(2):
                    h = 2 * hp + hx
                    uv = u_hx[hx].rearrange("p (r c) -> p r c", c=65)
                    rz = rzp.tile([P, 4, 1], F32, tag="rz", name="rz")
                    nc.vector.reciprocal(rz, uv[:, :, 64:65])
                    for it in range(4):
                        nc.vector.tensor_scalar(
                            o_sb[:, ib * 4 + it, h * DK : (h + 1) * DK],
                            uv[:, it, 0:DK],
                            rz[:, it],
                            None,
                            op0=MULT,
                        )
                if hp == 0 and ib == NIB - 1:
                    # last i-block: transpose the hp0 heads during the final
                    # sweep so less phase-C work serializes after the last exp
                    work.extend(
                        [lambda pair=pair: transp2(NIB - 1, 0, pair) for pair in range(2)]
                    )
                elif hp == 1:
                    items = phase_c(ib) if ib < NIB - 1 else phase_c_late(ib)
                    work.extend(items)

            for idx in range(len(seq) + 1):
                if idx < len(seq):
                    emit_st_exp(idx)
                if idx >= 1:
                    emit_pv(idx - 1)
                drain(idx)

            while work or light:
                (light or work).popleft()()

            if dbg:
                for t in range(2):
                    qsrc = qt_sb[t].bitcast(qk_dt) if st_dtype == F32R else qt_sb[t]
                    ksrc = kt_sb[t].bitcast(qk_dt) if st_dtype == F32R else kt_sb[t]
                    nc.sync.dma_start(out=d_qt.ap()[t], in_=qsrc)
                    nc.sync.dma_start(out=d_kt.ap()[t], in_=ksrc)
                nc.sync.dma_start(out=d_va.ap(), in_=vaug)
                nc.sync.dma_start(out=d_o.ap(), in_=o_sb.bitcast(F32))

    nc.compile()
    return nc


def _get_nc():
    global _NC_CACHE
    if _NC_CACHE is None:
        _NC_CACHE = _build()
    return _NC_CACHE


def _in_maps(query, key, value, wq, wk, wv, wo, bq, bk):
    maps = []
    for c in range(8):
        b, g = divmod(c, 4)
        sl = slice(g * DC, (g + 1) * DC)
        maps.append(
            {
                "xq_t": np.ascontiguousarray(query[:, b, :].T),
                "xk_t": np.ascontiguousarray(key[:, b, :].T),
                "xv_t": np.ascontiguousarray(value[:, b, :].T),
                "wq_t": np.ascontiguousarray(wq[sl, :].T),
                "wk_t": np.ascontiguousarray(wk[sl, :].T),
                "wv_t": np.ascontiguousarray(wv[sl, :].T),
                "wo_t": np.ascontiguousarray(wo[:, sl].T),
                "bq_s": np.ascontiguousarray(bq[sl].reshape(2, P).T),
                "bk_s": np.ascontiguousarray(bk[sl].reshape(2, P).T),
            }
        )
    return maps


def kernel(
    query, key, value, wq, bq, wk, bk, wv, bv, wo, bo, **_kw
) -> np.ndarray:
    query = np.asarray(query, np.float32)
    key = np.asarray(key, np.float32)
    value = np.asarray(value, np.float32)
    wq = np.asarray(wq, np.float32)
    wk = np.asarray(wk, np.float32)
    wv = np.asarray(wv, np.float32)
    wo = np.asarray(wo, np.float32)
    bq = np.asarray(bq, np.float32)
    bk = np.asarray(bk, np.float32)
    bv = np.asarray(bv, np.float32)
    bo = np.asarray(bo, np.float32)

    nc = _get_nc()
    res = run_bass_kernel_spmd(
        nc, _in_maps(query, key, value, wq, wk, wv, wo, bq, bk),
        core_ids=list(range(8)),
    )

    out = np.zeros((S, B, D), np.float32)
    for c in range(8):
        out[:, c // 4, :] += res.results[c]["y"]
    out += bo + wo @ bv
    return out


# revision 71
# speedup vs baseline: 1.0009x; 1.0002x over previous
"""Multi-head attention (S=2048, B=2, D=1024, H=16) on 8 Trainium2 NeuronCores.

Sharding: batch x head-group. Core c handles batch c//4 and heads
[4*(c%4), 4*(c%4)+4). Each core computes its 4 heads' Q/K/V projections,
attention, and a partial output projection (row-parallel Wo); the host sums
the 4 partials per batch and adds the bias terms (bo and the exact wo@bv
correction; softmax rows sum to 1 so bv folds out of the attention).

Device-side structure (per core):
  - inputs pre-transposed on host: xq_t/xk_t/xv_t (D, S) so the projection
    contraction dim (d) lands on SBUF partitions.
  - QT/KT (dk-major, 2 tiles of (128, S)): one head pair per tile, f32r.
  - scores computed transposed, ST = (j, i), via fp32r matmuls; the two heads
    of a pair run concurrently in disjoint PE row groups (K=dk=64 each).
  - softmax: exp on ScalarE straight out of PSUM with the 1/sqrt(dk) scale
    folded into the activation; no max subtraction (scores are O(1) here);
    normalization is deferred past P@V by appending a ones column to V so the
    PE accumulates the denominator Z next to U = exp(S^T).T @ V.
  - O = U * (1/Z) per query row (per-partition scalars on DVE), PE-transposed,
    then the output projection runs in f32r.
  - projections are emitted column-block-wise through a worklist drained
    inside the attention J-loop so DMA/PE/ACT/DVE overlap end to end; the
    first i-block's two head pairs are interleaved in 4-tick blocks so the
    exp stream has twice the schedulable work per K/V column-block arrival
    during the DMA-bound fill window.
"""

import sys

sys.path.insert(0, "/opt/trn_rl_repo")

from collections import deque

import numpy as np

import concourse.bass as bass
import concourse.tile as tile
from concourse import bacc, mybir
from concourse.bass_utils import run_bass_kernel_spmd
from concourse.masks import make_identity

S = 2048
B = 2
D = 1024
H = 16
DK = 64
G = 4            # heads per core
DC = G * DK      # 256 per-core head dims
SCALE = 1.0 / np.sqrt(DK)  # 0.125
P = 128
NJ = S // P      # 16 key chunks
NIT = S // P     # 16 query tiles
NIB = 4          # i blocks of 512
IB = S // NIB    # 512
ND = D // P      # 8 contraction chunks for projections

F32 = mybir.dt.float32
F32R = mybir.dt.float32r
BF16 = mybir.dt.bfloat16
FP16 = mybir.dt.float16
EXP = mybir.ActivationFunctionType.Exp
ADD = mybir.AluOpType.add
MULT = mybir.AluOpType.mult

_NC_CACHE = None


def _build(dbg=False, st_dtype=F32R):
    nc = bacc.Bacc("TRN2", target_bir_lowering=False, debug=False)

    xq_t = nc.dram_tensor("xq_t", [D, S], F32R, kind="ExternalInput")
    xk_t = nc.dram_tensor("xk_t", [D, S], F32R, kind="ExternalInput")
    xv_t = nc.dram_tensor("xv_t", [D, S], F32R, kind="ExternalInput")
    wq_t = nc.dram_tensor("wq_t", [D, DC], F32R, kind="ExternalInput")
    wk_t = nc.dram_tensor("wk_t", [D, DC], F32R, kind="ExternalInput")
    wv_t = nc.dram_tensor("wv_t", [D, DC], F32R, kind="ExternalInput")
    wo_t = nc.dram_tensor("wo_t", [DC, D], F32R, kind="ExternalInput")
    bq_s = nc.dram_tensor("bq_s", [P, 2], F32, kind="ExternalInput")
    bk_s = nc.dram_tensor("bk_s", [P, 2], F32, kind="ExternalInput")
    y = nc.dram_tensor("y", [S, D], F32, kind="ExternalOutput")
    if dbg:
        qk_dt = F32 if st_dtype == F32R else st_dtype
        d_qt = nc.dram_tensor("d_qt", [2, P, S], qk_dt, kind="ExternalOutput")
        d_kt = nc.dram_tensor("d_kt", [2, P, S], qk_dt, kind="ExternalOutput")
        d_va = nc.dram_tensor("d_va", [P, NJ, G * 65], FP16, kind="ExternalOutput")
        d_o = nc.dram_tensor("d_o", [P, NIT, DC], F32, kind="ExternalOutput")
        d_et = nc.dram_tensor("d_et", [P, 2 * IB], FP16, kind="ExternalOutput")
        d_u = nc.dram_tensor("d_u", [2, P, 4 * 65], F32, kind="ExternalOutput")

    with tile.TileContext(nc) as tc:
        with (
            tc.tile_pool(name="persist", bufs=1) as persist,
            tc.tile_pool(name="xs", bufs=24) as xs,
            tc.tile_pool(name="ps", bufs=2, space="PSUM") as psp,   # st/proj shared
            tc.tile_pool(name="up", bufs=4, space="PSUM") as up,    # U pairs + phase C
            tc.tile_pool(name="et", bufs=8) as etp,
            tc.tile_pool(name="rz", bufs=2) as rzp,
            tc.tile_pool(name="ysb", bufs=2) as ysb,
        ):
            # ---- persistent SBUF (DMAs ordered by first use) ----
            wq_sb = persist.tile([P, ND, DC], F32R)
            wk_sb = persist.tile([P, ND, DC], F32R)
            wv_sb = persist.tile([P, ND, DC], F32R)
            bq_sb = persist.tile([P, 2], F32)
            bk_sb = persist.tile([P, 2], F32)
            nc.sync.dma_start(out=wk_sb, in_=wk_t.ap().rearrange("(c p) m -> p c m", p=P))
            nc.sync.dma_start(out=bk_sb, in_=bk_s.ap())
            nc.sync.dma_start(out=bq_sb, in_=bq_s.ap())
            woc_sb = persist.tile([P, 2, D], F32R)

            qt_sb = [persist.tile([P, S], st_dtype, tag=f"qt{t}", name=f"qt{t}") for t in range(2)]
            kt_sb = [persist.tile([P, S], st_dtype, tag=f"kt{t}", name=f"kt{t}") for t in range(2)]
            vaug = persist.tile([P, NJ, G * 65], FP16)
            for h in range(G):
                nc.vector.memset(vaug[:, :, h * 65 + 64 : h * 65 + 65], 1.0)
            o_sb = persist.tile([P, NIT, DC], F32R)
            ot_sb = [persist.tile([P, S], F32R, tag=f"ot{t}", name=f"ot{t}") for t in range(2)]
            ident_f = persist.tile([P, P], F32)
            make_identity(nc, ident_f)
            ident = persist.tile([P, P], F32R)
            nc.vector.tensor_copy(ident, ident_f)

            # ---- work items: column-block DMA + projections, phase-C steps ----
            slabs = {}       # (tensor_key, cb) -> list of 8 (128, IB) tiles
            emitted = set()  # work-item keys already emitted

            HW_ = IB // 2

            def dma_slab(key, xt, cb):
                tiles = []
                for dc in range(ND):
                    t = xs.tile([P, IB], F32R, tag="x", name="x")
                    nc.sync.dma_start(
                        out=t, in_=xt.ap()[dc * P : (dc + 1) * P, cb * IB : (cb + 1) * IB]
                    )
                    tiles.append(t)
                slabs[(key, cb)] = tiles
                emitted.add(("dma", key, cb))

            def proj_qk(key, cb, mt, half, w_sb, b_sb, out_tiles):
                # half-width (256-col) groups keep the PSUM slot held for
                # less than one exp tick, so background projection work
                # never stalls the attention pipeline.
                ps = psp.tile([P, HW_], F32, tag="ps", name="ps")
                x_tiles = slabs[(key, cb)]
                for dc in range(ND):
                    nc.tensor.matmul(
                        ps,
                        w_sb[:, dc, mt * P : (mt + 1) * P],
                        x_tiles[dc][:, half * HW_ : (half + 1) * HW_],
                        start=(dc == 0),
                        stop=(dc == ND - 1),
                    )
                nc.vector.tensor_scalar(
                    out_tiles[mt][:, cb * IB + half * HW_ : cb * IB + (half + 1) * HW_],
                    ps,
                    b_sb[:, mt : mt + 1],
                    None,
                    op0=ADD,
                )
                emitted.add((key, cb, mt, half))

            def proj_v(cb, jq):
                # jq: 0..3 within the column block; global j chunk jt
                jt = cb * 4 + jq
                ps = psp.tile([P, DC], F32, tag="ps", name="ps")
                x_tiles = slabs[("v", cb)]
                for dc in range(ND):
                    nc.tensor.matmul(
                        ps,
                        x_tiles[dc][:, jq * P : (jq + 1) * P],
                        wv_sb[:, dc, :],
                        start=(dc == 0),
                        stop=(dc == ND - 1),
                    )
                nc.vector.tensor_copy(
                    vaug[:, jt].rearrange("p (h c) -> p h c", h=G)[:, :, 0:DK],
                    ps.rearrange("p (h c) -> p h c", h=G),
                )
                emitted.add(("v", cb, jq))

            def oproj(it):
                ys = ysb.tile([P, D], F32, tag="ysb", name="ysb")
                for nh in range(2):
                    yp = up.tile([P, IB], F32, tag="u", name="yp")
                    for mt in range(2):
                        nc.tensor.matmul(
                            yp,
                            ot_sb[mt][:, it * P : (it + 1) * P],
                            woc_sb[:, mt, nh * IB : (nh + 1) * IB],
                            start=(mt == 0),
                            stop=(mt == 1),
                        )
                    nc.vector.tensor_copy(ys[:, nh * IB : (nh + 1) * IB], yp)
                nc.sync.dma_start(out=y.ap()[it * P : (it + 1) * P, :], in_=ys)

            def phase_c(ib):
                items = [
                    lambda mt=mt, pair=pair: transp2(ib, mt, pair)
                    for mt in range(2)
                    for pair in range(2)
                ]
                items += [lambda it=it: oproj(it) for it in range(ib * 4, ib * 4 + 4)]
                return items

            def phase_c_late(ib):
                # hp0 transposes were already queued during the last sweep
                items = [lambda pair=pair: transp2(ib, 1, pair) for pair in range(2)]
                items += [lambda it=it: oproj(it) for it in range(ib * 4, ib * 4 + 4)]
                return items

            def transp2(ib, mt, pair):
                for it in range(ib * 4 + 2 * pair, ib * 4 + 2 * pair + 2):
                    tp = up.tile([P, P], F32R, tag="u", name="tp")
                    nc.tensor.transpose(
                        tp, o_sb[:, it, mt * P : (mt + 1) * P], ident
                    )
                    nc.vector.tensor_copy(
                        ot_sb[mt][:, it * P : (it + 1) * P], tp
                    )

            work = deque()    # psum-holding items: paced to alternate ticks
            light = deque()   # DMA/staging items: drained every tick

            def drain(tick):
                if light:
                    light.popleft()()
                if work and (tick % 2 == 0 or not light):
                    work.popleft()()

            def drain_until(key):
                while key not in emitted:
                    assert work or light, f"work exhausted before {key}"
                    if light:
                        light.popleft()()
                    elif work:
                        work.popleft()()

            # column block 0 emitted up front (the pipeline fill); cb0's
            # V-projection drains inside the first J sweep instead of
            # stalling the PE stream ahead of the first score matmul.
            # column block 0 emitted up front (the pipeline fill); cb0's
            # V-projection and the hp1 (mt=1) projections drain inside the
            # first J sweep instead of stalling ahead of the first score MM.
            dma_slab("k", xk_t, 0)
            proj_qk("k", 0, 0, 0, wk_sb, bk_sb, kt_sb)
            proj_qk("k", 0, 0, 1, wk_sb, bk_sb, kt_sb)
            nc.sync.dma_start(out=wq_sb, in_=wq_t.ap().rearrange("(c p) m -> p c m", p=P))
            dma_slab("q", xq_t, 0)
            proj_qk("q", 0, 0, 0, wq_sb, bq_sb, qt_sb)
            proj_qk("q", 0, 0, 1, wq_sb, bq_sb, qt_sb)
            nc.sync.dma_start(out=wv_sb, in_=wv_t.ap().rearrange("(c p) m -> p c m", p=P))
            dma_slab("v", xv_t, 0)

            def load_woc():
                nc.sync.dma_start(
                    out=woc_sb, in_=wo_t.ap().rearrange("(t p) n -> p t n", p=P)
                )

            work.append(lambda: proj_qk("k", 0, 1, 0, wk_sb, bk_sb, kt_sb))
            work.append(lambda: proj_qk("k", 0, 1, 1, wk_sb, bk_sb, kt_sb))
            work.append(lambda: proj_qk("q", 0, 1, 0, wq_sb, bq_sb, qt_sb))
            work.append(lambda: proj_qk("q", 0, 1, 1, wq_sb, bq_sb, qt_sb))
            for jq in range(4):
                work.append(lambda jq=jq: proj_v(0, jq))
            for cb in range(1, NIB):
                light.append(lambda cb=cb: dma_slab("k", xk_t, cb))
                for half in range(2):
                    work.append(lambda cb=cb, half=half: proj_qk("k", cb, 0, half, wk_sb, bk_sb, kt_sb))
                    work.append(lambda cb=cb, half=half: proj_qk("k", cb, 1, half, wk_sb, bk_sb, kt_sb))
                light.append(lambda cb=cb: dma_slab("v", xv_t, cb))
                for jq in range(4):
                    work.append(lambda cb=cb, jq=jq: proj_v(cb, jq))
            light.append(load_woc)
            for cb in range(1, NIB):
                light.append(lambda cb=cb: dma_slab("q", xq_t, cb))
                for half in range(2):
                    work.append(lambda cb=cb, half=half: proj_qk("q", cb, 0, half, wq_sb, bq_sb, qt_sb))
                    work.append(lambda cb=cb, half=half: proj_qk("q", cb, 1, half, wq_sb, bq_sb, qt_sb))

            # ---- attention: flat software pipeline over (ib, hp, J) ticks ----
            # scores+exp for tick t are emitted one tick ahead of tick t-1's
            # P@V matmuls, so the exp stream never waits on a pair boundary.
            seq = [(0, hp, jblk * 4 + J) for jblk in range(4) for hp in range(2) for J in range(4)]
            seq += [(ib, hp, J) for ib in range(1, NIB) for hp in range(2) for J in range(NJ)]
            u_tiles = {}   # (ib, hp) -> [uA, uB]
            et_tiles = {}  # tick index -> et tile

            def emit_st_exp(idx):
                ib, hp, J = seq[idx]
                if J == 0:
                    drain_until(("q", ib, hp, 0))
                    drain_until(("q", ib, hp, 1))
                if ib == 0:
                    drain_until(("k", J // 4, hp, 0))
                    drain_until(("k", J // 4, hp, 1))
                st = psp.tile([P, 2 * IB], F32, tag="ps", name="st")
                for hx in range(2):
                    nc.tensor.matmul(
                        st[:, hx * IB : (hx + 1) * IB],
                        kt_sb[hp][hx * DK : (hx + 1) * DK, J * P : (J + 1) * P],
                        qt_sb[hp][hx * DK : (hx + 1) * DK, ib * IB : (ib + 1) * IB],
                        start=True,
                        stop=True,
                        tile_position=(hx * DK, 0),
                    )
                et = etp.tile([P, 2 * IB], FP16, tag="et", name="et")
                nc.scalar.activation(et, st, EXP, scale=float(SCALE))
                et_tiles[idx] = et
                if dbg and idx == 0:
                    nc.sync.dma_start(out=d_et.ap(), in_=et)

            def emit_pv(idx):
                ib, hp, J = seq[idx]
                if J == 0:
                    u_tiles[(ib, hp)] = [
                        up.tile([P, 4 * 65], F32, tag="u", name="u") for _ in range(2)
                    ]
                if ib == 0 and hp == 0:
                    drain_until(("v", J // 4, J % 4))
                et = et_tiles.pop(idx)
                u_hx = u_tiles[(ib, hp)]
                for hx in range(2):
                    h = 2 * hp + hx
                    for it in range(4):
                        # start clears has_written for the WHOLE bank: only the
                        # first matmul into this U bank may set it.
                        nc.tensor.matmul(
                            u_hx[hx][:, it * 65 : (it + 1) * 65],
                            et[:, hx * IB + it * P : hx * IB + (it + 1) * P],
                            vaug[:, J, h * 65 : (h + 1) * 65],
                            start=(J == 0 and it == 0),
                            stop=(J == NJ - 1 and it == 3),
                            skip_group_check=True,
                        )
                if J == NJ - 1:
                    finish_pair(ib, hp)

            def finish_pair(ib, hp):
                u_hx = u_tiles.pop((ib, hp))
                if dbg and ib == 0 and hp == 0:
                    for hx in range(2):
                        du_sb = ysb.tile([P, 4 * 65], F32, tag="du", name="du")
                        nc.vector.tensor_copy(du_sb, u_hx[hx])
                        nc.sync.dma_start(out=d_u.ap()[hx], in_=du_sb)
                for hx in range(2):
                    h = 2 * hp + hx
                    uv = u_hx[hx].rearrange("p (r c) -> p r c", c=65)
                    rz = rzp.tile([P, 4, 1], F32, tag="rz", name="rz")
                    nc.vector.reciprocal(rz, uv[:, :, 64:65])
                    for it in range(4):
                        nc.vector.tensor_scalar(
                            o_sb[:, ib * 4 + it, h * DK : (h + 1) * DK],
                            uv[:, it, 0:DK],
                            rz[:, it],
                            None,
                            op0=MULT,
                        )
                if hp == 0 and ib == NIB - 1:
                    # last i-block: transpose the hp0 heads during the final
                    # sweep so less phase-C work serializes after the last exp
                    work.extend(
                        [lambda pair=pair: transp2(NIB - 1, 0, pair) for pair in range(2)]
                    )
                elif hp == 1:
                    items = phase_c(ib) if ib < NIB - 1 else phase_c_late(ib)
                    work.extend(items)

            for idx in range(len(seq) + 1):
                if idx < len(seq):
                    emit_st_exp(idx)
                if idx >= 1:
                    emit_pv(idx - 1)
                drain(idx)

            while work or light:
                (light or work).popleft()()

            if dbg:
                for t in range(2):
                    qsrc = qt_sb[t].bitcast(qk_dt) if st_dtype == F32R else qt_sb[t]
                    ksrc = kt_sb[t].bitcast(qk_dt) if st_dtype == F32R else kt_sb[t]
                    nc.sync.dma_start(out=d_qt.ap()[t], in_=qsrc)
                    nc.sync.dma_start(out=d_kt.ap()[t], in_=ksrc)
                nc.sync.dma_start(out=d_va.ap(), in_=vaug)
                nc.sync.dma_start(out=d_o.ap(), in_=o_sb.bitcast(F32))

    nc.compile()
    return nc


def _get_nc():
    global _NC_CACHE
    if _NC_CACHE is None:
        _NC_CACHE = _build()
    return _NC_CACHE


def _in_maps(query, key, value, wq, wk, wv, wo, bq, bk):
    maps = []
    for c in range(8):
        b, g = divmod(c, 4)
        sl = slice(g * DC, (g + 1) * DC)
        maps.append(
            {
                "xq_t": np.ascontiguousarray(query[:, b, :].T),
                "xk_t": np.ascontiguousarray(key[:, b, :].T),
                "xv_t": np.ascontiguousarray(value[:, b, :].T),
                "wq_t": np.ascontiguousarray(wq[sl, :].T),
                "wk_t": np.ascontiguousarray(wk[sl, :].T),
                "wv_t": np.ascontiguousarray(wv[sl, :].T),
                "wo_t": np.ascontiguousarray(wo[:, sl].T),
                "bq_s": np.ascontiguousarray(bq[sl].reshape(2, P).T),
                "bk_s": np.ascontiguousarray(bk[sl].reshape(2, P).T),
            }
        )
    return maps


def kernel(
    query, key, value, wq, bq, wk, bk, wv, bv, wo, bo, **_kw
) -> np.ndarray:
    query = np.asarray(query, np.float32)
    key = np.asarray(key, np.float32)
    value = np.asarray(value, np.float32)
    wq = np.asarray(wq, np.float32)
    wk = np.asarray(wk, np.float32)
    wv = np.asarray(wv, np.float32)
    wo = np.asarray(wo, np.float32)
    bq = np.asarray(bq, np.float32)
    bk = np.asarray(bk, np.float32)
    bv = np.asarray(bv, np.float32)
    bo = np.asarray(bo, np.float32)

    nc = _get_nc()
    res = run_bass_kernel_spmd(
        nc, _in_maps(query, key, value, wq, wk, wv, wo, bq, bk),
        core_ids=list(range(8)),
    )

    out = np.zeros((S, B, D), np.float32)
    for c in range(8):
        out[:, c // 4, :] += res.results[c]["y"]
    out += bo + wo @ bv
    return out


# revision 74
# speedup vs baseline: 1.0025x; 1.0016x over previous
"""Multi-head attention (S=2048, B=2, D=1024, H=16) on 8 Trainium2 NeuronCores.

Sharding: batch x head-group. Core c handles batch c//4 and heads
[4*(c%4), 4*(c%4)+4). Each core computes its 4 heads' Q/K/V projections,
attention, and a partial output projection (row-parallel Wo); the host sums
the 4 partials per batch and adds the bias terms (bo and the exact wo@bv
correction; softmax rows sum to 1 so bv folds out of the attention).

Device-side structure (per core):
  - inputs pre-transposed on host: xq_t/xk_t/xv_t (D, S) so the projection
    contraction dim (d) lands on SBUF partitions.
  - QT/KT (dk-major, 2 tiles of (128, S)): one head pair per tile, f32r.
  - scores computed transposed, ST = (j, i), via fp32r matmuls; the two heads
    of a pair run concurrently in disjoint PE row groups (K=dk=64 each).
  - softmax: exp on ScalarE straight out of PSUM with the 1/sqrt(dk) scale
    folded into the activation; no max subtraction (scores are O(1) here);
    normalization is deferred past P@V by appending a ones column to V so the
    PE accumulates the denominator Z next to U = exp(S^T).T @ V.
  - O = U * (1/Z) per query row (per-partition scalars on DVE), PE-transposed,
    then the output projection runs in f32r.
  - projections are emitted column-block-wise through a worklist drained
    inside the attention J-loop so DMA/PE/ACT/DVE overlap end to end; the
    first i-block's two head pairs are interleaved in 4-tick blocks so the
    exp stream has twice the schedulable work per K/V column-block arrival
    during the DMA-bound fill window.
"""

import sys

sys.path.insert(0, "/opt/trn_rl_repo")

from collections import deque

import numpy as np

import concourse.bass as bass
import concourse.tile as tile
from concourse import bacc, mybir
from concourse.bass_utils import run_bass_kernel_spmd
from concourse.masks import make_identity

S = 2048
B = 2
D = 1024
H = 16
DK = 64
G = 4            # heads per core
DC = G * DK      # 256 per-core head dims
SCALE = 1.0 / np.sqrt(DK)  # 0.125
P = 128
NJ = S // P      # 16 key chunks
NIT = S // P     # 16 query tiles
NIB = 4          # i blocks of 512
IB = S // NIB    # 512
ND = D // P      # 8 contraction chunks for projections

F32 = mybir.dt.float32
F32R = mybir.dt.float32r
BF16 = mybir.dt.bfloat16
FP16 = mybir.dt.float16
EXP = mybir.ActivationFunctionType.Exp
ADD = mybir.AluOpType.add
MULT = mybir.AluOpType.mult

_NC_CACHE = None


def _build(dbg=False, st_dtype=F32R):
    nc = bacc.Bacc("TRN2", target_bir_lowering=False, debug=False)

    xq_t = nc.dram_tensor("xq_t", [D, S], F32R, kind="ExternalInput")
    xk_t = nc.dram_tensor("xk_t", [D, S], F32R, kind="ExternalInput")
    xv_t = nc.dram_tensor("xv_t", [D, S], F32R, kind="ExternalInput")
    wq_t = nc.dram_tensor("wq_t", [D, DC], F32R, kind="ExternalInput")
    wk_t = nc.dram_tensor("wk_t", [D, DC], F32R, kind="ExternalInput")
    wv_t = nc.dram_tensor("wv_t", [D, DC], F32R, kind="ExternalInput")
    wo_t = nc.dram_tensor("wo_t", [DC, D], F32R, kind="ExternalInput")
    bq_s = nc.dram_tensor("bq_s", [P, 2], F32, kind="ExternalInput")
    bk_s = nc.dram_tensor("bk_s", [P, 2], F32, kind="ExternalInput")
    y = nc.dram_tensor("y", [S, D], F32, kind="ExternalOutput")
    if dbg:
        qk_dt = F32 if st_dtype == F32R else st_dtype
        d_qt = nc.dram_tensor("d_qt", [2, P, S], qk_dt, kind="ExternalOutput")
        d_kt = nc.dram_tensor("d_kt", [2, P, S], qk_dt, kind="ExternalOutput")
        d_va = nc.dram_tensor("d_va", [P, NJ, G * 65], FP16, kind="ExternalOutput")
        d_o = nc.dram_tensor("d_o", [P, NIT, DC], F32, kind="ExternalOutput")
        d_et = nc.dram_tensor("d_et", [P, 2 * IB], FP16, kind="ExternalOutput")
        d_u = nc.dram_tensor("d_u", [2, P, 4 * 65], F32, kind="ExternalOutput")

    with tile.TileContext(nc) as tc:
        with (
            tc.tile_pool(name="persist", bufs=1) as persist,
            tc.tile_pool(name="xs", bufs=24) as xs,
            tc.tile_pool(name="ps", bufs=2, space="PSUM") as psp,   # st/proj shared
            tc.tile_pool(name="up", bufs=4, space="PSUM") as up,    # U pairs + phase C
            tc.tile_pool(name="et", bufs=8) as etp,
            tc.tile_pool(name="rz", bufs=2) as rzp,
            tc.tile_pool(name="ysb", bufs=2) as ysb,
        ):
            # ---- persistent SBUF (DMAs ordered by first use) ----
            wq_sb = persist.tile([P, ND, DC], F32R)
            wk_sb = persist.tile([P, ND, DC], F32R)
            wv_sb = persist.tile([P, ND, DC], F32R)
            bq_sb = persist.tile([P, 2], F32)
            bk_sb = persist.tile([P, 2], F32)
            nc.sync.dma_start(out=wk_sb, in_=wk_t.ap().rearrange("(c p) m -> p c m", p=P))
            nc.sync.dma_start(out=bk_sb, in_=bk_s.ap())
            nc.sync.dma_start(out=bq_sb, in_=bq_s.ap())
            woc_sb = persist.tile([P, 2, D], F32R)

            qt_sb = [persist.tile([P, S], st_dtype, tag=f"qt{t}", name=f"qt{t}") for t in range(2)]
            kt_sb = [persist.tile([P, S], st_dtype, tag=f"kt{t}", name=f"kt{t}") for t in range(2)]
            vaug = persist.tile([P, NJ, G * 65], FP16)
            for h in range(G):
                nc.vector.memset(vaug[:, :, h * 65 + 64 : h * 65 + 65], 1.0)
            o_sb = persist.tile([P, NIT, DC], F32R)
            ot_sb = [persist.tile([P, S], F32R, tag=f"ot{t}", name=f"ot{t}") for t in range(2)]
            ident_f = persist.tile([P, P], F32)
            make_identity(nc, ident_f)
            ident = persist.tile([P, P], F32R)
            nc.vector.tensor_copy(ident, ident_f)

            # ---- work items: column-block DMA + projections, phase-C steps ----
            slabs = {}       # (tensor_key, cb) -> list of 8 (128, IB) tiles
            emitted = set()  # work-item keys already emitted

            HW_ = IB // 2

            def dma_slab(key, xt, cb):
                tiles = []
                for dc in range(ND):
                    t = xs.tile([P, IB], F32R, tag="x", name="x")
                    nc.sync.dma_start(
                        out=t, in_=xt.ap()[dc * P : (dc + 1) * P, cb * IB : (cb + 1) * IB]
                    )
                    tiles.append(t)
                slabs[(key, cb)] = tiles
                emitted.add(("dma", key, cb))

            def proj_qk(key, cb, mt, half, w_sb, b_sb, out_tiles):
                # half-width (256-col) groups keep the PSUM slot held for
                # less than one exp tick, so background projection work
                # never stalls the attention pipeline.
                ps = psp.tile([P, HW_], F32, tag="ps", name="ps")
                x_tiles = slabs[(key, cb)]
                for dc in range(ND):
                    nc.tensor.matmul(
                        ps,
                        w_sb[:, dc, mt * P : (mt + 1) * P],
                        x_tiles[dc][:, half * HW_ : (half + 1) * HW_],
                        start=(dc == 0),
                        stop=(dc == ND - 1),
                    )
                nc.vector.tensor_scalar(
                    out_tiles[mt][:, cb * IB + half * HW_ : cb * IB + (half + 1) * HW_],
                    ps,
                    b_sb[:, mt : mt + 1],
                    None,
                    op0=ADD,
                )
                emitted.add((key, cb, mt, half))

            def proj_v(cb, jq):
                # jq: 0..3 within the column block; global j chunk jt
                jt = cb * 4 + jq
                ps = psp.tile([P, DC], F32, tag="ps", name="ps")
                x_tiles = slabs[("v", cb)]
                for dc in range(ND):
                    nc.tensor.matmul(
                        ps,
                        x_tiles[dc][:, jq * P : (jq + 1) * P],
                        wv_sb[:, dc, :],
                        start=(dc == 0),
                        stop=(dc == ND - 1),
                    )
                nc.vector.tensor_copy(
                    vaug[:, jt].rearrange("p (h c) -> p h c", h=G)[:, :, 0:DK],
                    ps.rearrange("p (h c) -> p h c", h=G),
                )
                emitted.add(("v", cb, jq))

            def oproj(it):
                ys = ysb.tile([P, D], F32, tag="ysb", name="ysb")
                for nh in range(2):
                    yp = up.tile([P, IB], F32, tag="u", name="yp")
                    for mt in range(2):
                        nc.tensor.matmul(
                            yp,
                            ot_sb[mt][:, it * P : (it + 1) * P],
                            woc_sb[:, mt, nh * IB : (nh + 1) * IB],
                            start=(mt == 0),
                            stop=(mt == 1),
                        )
                    nc.vector.tensor_copy(ys[:, nh * IB : (nh + 1) * IB], yp)
                nc.sync.dma_start(out=y.ap()[it * P : (it + 1) * P, :], in_=ys)

            def phase_c(ib):
                items = [
                    lambda mt=mt, pair=pair: transp2(ib, mt, pair)
                    for mt in range(2)
                    for pair in range(2)
                ]
                items += [lambda it=it: oproj(it) for it in range(ib * 4, ib * 4 + 4)]
                return items

            def phase_c_late(ib):
                # hp0 transposes were already queued during the last sweep;
                # start each it-pair's output projection right after its own
                # mt1 transpose so the tail chain is as short as possible
                items = []
                for pair in range(2):
                    items.append(lambda pair=pair: transp2(ib, 1, pair))
                    for it in range(ib * 4 + 2 * pair, ib * 4 + 2 * pair + 2):
                        items.append(lambda it=it: oproj(it))
                return items

            def transp2(ib, mt, pair):
                for it in range(ib * 4 + 2 * pair, ib * 4 + 2 * pair + 2):
                    tp = up.tile([P, P], F32R, tag="u", name="tp")
                    nc.tensor.transpose(
                        tp, o_sb[:, it, mt * P : (mt + 1) * P], ident
                    )
                    nc.vector.tensor_copy(
                        ot_sb[mt][:, it * P : (it + 1) * P], tp
                    )

            work = deque()    # psum-holding items: paced to alternate ticks
            light = deque()   # DMA/staging items: drained every tick

            def drain(tick):
                if light:
                    light.popleft()()
                if work and (tick % 2 == 0 or not light):
                    work.popleft()()

            def drain_until(key):
                while key not in emitted:
                    assert work or light, f"work exhausted before {key}"
                    if light:
                        light.popleft()()
                    elif work:
                        work.popleft()()

            # column block 0 emitted up front (the pipeline fill); cb0's
            # V-projection drains inside the first J sweep instead of
            # stalling the PE stream ahead of the first score matmul.
            # column block 0 emitted up front (the pipeline fill); cb0's
            # V-projection and the hp1 (mt=1) projections drain inside the
            # first J sweep instead of stalling ahead of the first score MM.
            dma_slab("k", xk_t, 0)
            proj_qk("k", 0, 0, 0, wk_sb, bk_sb, kt_sb)
            proj_qk("k", 0, 0, 1, wk_sb, bk_sb, kt_sb)
            nc.sync.dma_start(out=wq_sb, in_=wq_t.ap().rearrange("(c p) m -> p c m", p=P))
            dma_slab("q", xq_t, 0)
            proj_qk("q", 0, 0, 0, wq_sb, bq_sb, qt_sb)
            proj_qk("q", 0, 0, 1, wq_sb, bq_sb, qt_sb)
            nc.sync.dma_start(out=wv_sb, in_=wv_t.ap().rearrange("(c p) m -> p c m", p=P))
            dma_slab("v", xv_t, 0)

            def load_woc():
                nc.sync.dma_start(
                    out=woc_sb, in_=wo_t.ap().rearrange("(t p) n -> p t n", p=P)
                )

            work.append(lambda: proj_qk("k", 0, 1, 0, wk_sb, bk_sb, kt_sb))
            work.append(lambda: proj_qk("k", 0, 1, 1, wk_sb, bk_sb, kt_sb))
            work.append(lambda: proj_qk("q", 0, 1, 0, wq_sb, bq_sb, qt_sb))
            work.append(lambda: proj_qk("q", 0, 1, 1, wq_sb, bq_sb, qt_sb))
            for jq in range(4):
                work.append(lambda jq=jq: proj_v(0, jq))
            for cb in range(1, NIB):
                light.append(lambda cb=cb: dma_slab("k", xk_t, cb))
                for half in range(2):
                    work.append(lambda cb=cb, half=half: proj_qk("k", cb, 0, half, wk_sb, bk_sb, kt_sb))
                    work.append(lambda cb=cb, half=half: proj_qk("k", cb, 1, half, wk_sb, bk_sb, kt_sb))
                light.append(lambda cb=cb: dma_slab("v", xv_t, cb))
                for jq in range(4):
                    work.append(lambda cb=cb, jq=jq: proj_v(cb, jq))
            light.append(load_woc)
            for cb in range(1, NIB):
                light.append(lambda cb=cb: dma_slab("q", xq_t, cb))
                for half in range(2):
                    work.append(lambda cb=cb, half=half: proj_qk("q", cb, 0, half, wq_sb, bq_sb, qt_sb))
                    work.append(lambda cb=cb, half=half: proj_qk("q", cb, 1, half, wq_sb, bq_sb, qt_sb))

            # ---- attention: flat software pipeline over (ib, hp, J) ticks ----
            # scores+exp for tick t are emitted one tick ahead of tick t-1's
            # P@V matmuls, so the exp stream never waits on a pair boundary.
            seq = [(0, hp, jblk * 4 + J) for jblk in range(4) for hp in range(2) for J in range(4)]
            seq += [(ib, hp, J) for ib in range(1, NIB) for hp in range(2) for J in range(NJ)]
            u_tiles = {}   # (ib, hp) -> [uA, uB]
            et_tiles = {}  # tick index -> et tile

            def emit_st_exp(idx):
                ib, hp, J = seq[idx]
                if J == 0:
                    drain_until(("q", ib, hp, 0))
                    drain_until(("q", ib, hp, 1))
                if ib == 0:
                    drain_until(("k", J // 4, hp, 0))
                    drain_until(("k", J // 4, hp, 1))
                st = psp.tile([P, 2 * IB], F32, tag="ps", name="st")
                for hx in range(2):
                    nc.tensor.matmul(
                        st[:, hx * IB : (hx + 1) * IB],
                        kt_sb[hp][hx * DK : (hx + 1) * DK, J * P : (J + 1) * P],
                        qt_sb[hp][hx * DK : (hx + 1) * DK, ib * IB : (ib + 1) * IB],
                        start=True,
                        stop=True,
                        tile_position=(hx * DK, 0),
                    )
                et = etp.tile([P, 2 * IB], FP16, tag="et", name="et")
                nc.scalar.activation(et, st, EXP, scale=float(SCALE))
                et_tiles[idx] = et
                if dbg and idx == 0:
                    nc.sync.dma_start(out=d_et.ap(), in_=et)

            def emit_pv(idx):
                ib, hp, J = seq[idx]
                if J == 0:
                    u_tiles[(ib, hp)] = [
                        up.tile([P, 4 * 65], F32, tag="u", name="u") for _ in range(2)
                    ]
                if ib == 0 and hp == 0:
                    drain_until(("v", J // 4, J % 4))
                et = et_tiles.pop(idx)
                u_hx = u_tiles[(ib, hp)]
                for hx in range(2):
                    h = 2 * hp + hx
                    for it in range(4):
                        # start clears has_written for the WHOLE bank: only the
                        # first matmul into this U bank may set it.
                        nc.tensor.matmul(
                            u_hx[hx][:, it * 65 : (it + 1) * 65],
                            et[:, hx * IB + it * P : hx * IB + (it + 1) * P],
                            vaug[:, J, h * 65 : (h + 1) * 65],
                            start=(J == 0 and it == 0),
                            stop=(J == NJ - 1 and it == 3),
                            skip_group_check=True,
                        )
                if J == NJ - 1:
                    finish_pair(ib, hp)

            def finish_pair(ib, hp):
                u_hx = u_tiles.pop((ib, hp))
                if dbg and ib == 0 and hp == 0:
                    for hx in range(2):
                        du_sb = ysb.tile([P, 4 * 65], F32, tag="du", name="du")
                        nc.vector.tensor_copy(du_sb, u_hx[hx])
                        nc.sync.dma_start(out=d_u.ap()[hx], in_=du_sb)
                for hx in range(2):
                    h = 2 * hp + hx
                    uv = u_hx[hx].rearrange("p (r c) -> p r c", c=65)
                    rz = rzp.tile([P, 4, 1], F32, tag="rz", name="rz")
                    nc.vector.reciprocal(rz, uv[:, :, 64:65])
                    for it in range(4):
                        nc.vector.tensor_scalar(
                            o_sb[:, ib * 4 + it, h * DK : (h + 1) * DK],
                            uv[:, it, 0:DK],
                            rz[:, it],
                            None,
                            op0=MULT,
                        )
                if hp == 0 and ib == NIB - 1:
                    # last i-block: transpose the hp0 heads during the final
                    # sweep so less phase-C work serializes after the last exp
                    work.extend(
                        [lambda pair=pair: transp2(NIB - 1, 0, pair) for pair in range(2)]
                    )
                elif hp == 1:
                    items = phase_c(ib) if ib < NIB - 1 else phase_c_late(ib)
                    work.extend(items)

            for idx in range(len(seq) + 1):
                if idx < len(seq):
                    emit_st_exp(idx)
                if idx >= 1:
                    emit_pv(idx - 1)
                drain(idx)

            while work or light:
                (light or work).popleft()()

            if dbg:
                for t in range(2):
                    qsrc = qt_sb[t].bitcast(qk_dt) if st_dtype == F32R else qt_sb[t]
                    ksrc = kt_sb[t].bitcast(qk_dt) if st_dtype == F32R else kt_sb[t]
                    nc.sync.dma_start(out=d_qt.ap()[t], in_=qsrc)
                    nc.sync.dma_start(out=d_kt.ap()[t], in_=ksrc)
                nc.sync.dma_start(out=d_va.ap(), in_=vaug)
                nc.sync.dma_start(out=d_o.ap(), in_=o_sb.bitcast(F32))

    nc.compile()
    return nc


def _get_nc():
    global _NC_CACHE
    if _NC_CACHE is None:
        _NC_CACHE = _build()
    return _NC_CACHE


def _in_maps(query, key, value, wq, wk, wv, wo, bq, bk):
    maps = []
    for c in range(8):
        b, g = divmod(c, 4)
        sl = slice(g * DC, (g + 1) * DC)
        maps.append(
            {
                "xq_t": np.ascontiguousarray(query[:, b, :].T),
                "xk_t": np.ascontiguousarray(key[:, b, :].T),
                "xv_t": np.ascontiguousarray(value[:, b, :].T),
                "wq_t": np.ascontiguousarray(wq[sl, :].T),
                "wk_t": np.ascontiguousarray(wk[sl, :].T),
                "wv_t": np.ascontiguousarray(wv[sl, :].T),
                "wo_t": np.ascontiguousarray(wo[:, sl].T),
                "bq_s": np.ascontiguousarray(bq[sl].reshape(2, P).T),
                "bk_s": np.ascontiguousarray(bk[sl].reshape(2, P).T),
            }
        )
    return maps


def kernel(
    query, key, value, wq, bq, wk, bk, wv, bv, wo, bo, **_kw
) -> np.ndarray:
    query = np.asarray(query, np.float32)
    key = np.asarray(key, np.float32)
    value = np.asarray(value, np.float32)
    wq = np.asarray(wq, np.float32)
    wk = np.asarray(wk, np.float32)
    wv = np.asarray(wv, np.float32)
    wo = np.asarray(wo, np.float32)
    bq = np.asarray(bq, np.float32)
    bk = np.asarray(bk, np.float32)
    bv = np.asarray(bv, np.float32)
    bo = np.asarray(bo, np.float32)

    nc = _get_nc()
    res = run_bass_kernel_spmd(
        nc, _in_maps(query, key, value, wq, wk, wv, wo, bq, bk),
        core_ids=list(range(8)),
    )

    out = np.zeros((S, B, D), np.float32)
    for c in range(8):
        out[:, c // 4, :] += res.results[c]["y"]
    out += bo + wo @ bv
    return out


# revision 79
# speedup vs baseline: 1.0131x; 1.0105x over previous
"""Multi-head attention (S=2048, B=2, D=1024, H=16) on 8 Trainium2 NeuronCores.

Sharding: batch x head-group. Core c handles batch c//4 and heads
[4*(c%4), 4*(c%4)+4). Each core computes its 4 heads' Q/K/V projections,
attention, and a partial output projection (row-parallel Wo); the host sums
the 4 partials per batch and adds the bias terms (bo and the exact wo@bv
correction; softmax rows sum to 1 so bv folds out of the attention).

Device-side structure (per core):
  - inputs pre-transposed on host: xq_t/xk_t/xv_t (D, S) so the projection
    contraction dim (d) lands on SBUF partitions.
  - QT/KT (dk-major, 2 tiles of (128, S)): one head pair per tile, f32r.
  - scores computed transposed, ST = (j, i), via fp32r matmuls; the two heads
    of a pair run concurrently in disjoint PE row groups (K=dk=64 each).
  - softmax: exp on ScalarE straight out of PSUM with the 1/sqrt(dk) scale
    folded into the activation; no max subtraction (scores are O(1) here);
    normalization is deferred past P@V by appending a ones column to V so the
    PE accumulates the denominator Z next to U = exp(S^T).T @ V.
  - O = U * (1/Z) per query row (per-partition scalars on DVE), PE-transposed,
    then the output projection runs in f32r.
  - projections are emitted column-block-wise through a worklist drained
    inside the attention J-loop so DMA/PE/ACT/DVE overlap end to end; the
    first i-block's two head pairs are interleaved in 4-tick blocks so the
    exp stream has twice the schedulable work per K/V column-block arrival
    during the DMA-bound fill window.
"""

import sys

sys.path.insert(0, "/opt/trn_rl_repo")

from collections import deque

import numpy as np

import concourse.bass as bass
import concourse.tile as tile
from concourse import bacc, mybir
from concourse.bass_utils import run_bass_kernel_spmd
from concourse.masks import make_identity

S = 2048
B = 2
D = 1024
H = 16
DK = 64
G = 4            # heads per core
DC = G * DK      # 256 per-core head dims
SCALE = 1.0 / np.sqrt(DK)  # 0.125
P = 128
NJ = S // P      # 16 key chunks
NIT = S // P     # 16 query tiles
NIB = 4          # i blocks of 512
IB = S // NIB    # 512
ND = D // P      # 8 contraction chunks for projections

F32 = mybir.dt.float32
F32R = mybir.dt.float32r
BF16 = mybir.dt.bfloat16
FP16 = mybir.dt.float16
EXP = mybir.ActivationFunctionType.Exp
ADD = mybir.AluOpType.add
MULT = mybir.AluOpType.mult

_NC_CACHE = None


def _build(dbg=False, st_dtype=F32R):
    nc = bacc.Bacc("TRN2", target_bir_lowering=False, debug=False)

    xq_t = nc.dram_tensor("xq_t", [D, S], F32R, kind="ExternalInput")
    xk_t = nc.dram_tensor("xk_t", [D, S], F32R, kind="ExternalInput")
    xv_t = nc.dram_tensor("xv_t", [D, S], F32R, kind="ExternalInput")
    wq_t = nc.dram_tensor("wq_t", [D, DC], F32R, kind="ExternalInput")
    wk_t = nc.dram_tensor("wk_t", [D, DC], F32R, kind="ExternalInput")
    wv_t = nc.dram_tensor("wv_t", [D, DC], F32R, kind="ExternalInput")
    wo_t = nc.dram_tensor("wo_t", [DC, D], F32R, kind="ExternalInput")
    bq_s = nc.dram_tensor("bq_s", [P, 2], F32, kind="ExternalInput")
    bk_s = nc.dram_tensor("bk_s", [P, 2], F32, kind="ExternalInput")
    y = nc.dram_tensor("y", [S, D], F32, kind="ExternalOutput")
    if dbg:
        qk_dt = F32 if st_dtype == F32R else st_dtype
        d_qt = nc.dram_tensor("d_qt", [2, P, S], qk_dt, kind="ExternalOutput")
        d_kt = nc.dram_tensor("d_kt", [2, P, S], qk_dt, kind="ExternalOutput")
        d_va = nc.dram_tensor("d_va", [P, NJ, G * 65], FP16, kind="ExternalOutput")
        d_o = nc.dram_tensor("d_o", [P, NIT, DC], F32, kind="ExternalOutput")
        d_et = nc.dram_tensor("d_et", [P, 2 * IB], FP16, kind="ExternalOutput")
        d_u = nc.dram_tensor("d_u", [2, P, 4 * 65], F32, kind="ExternalOutput")

    with tile.TileContext(nc) as tc:
        with (
            tc.tile_pool(name="persist", bufs=1) as persist,
            tc.tile_pool(name="xs", bufs=24) as xs,
            tc.tile_pool(name="ps", bufs=2, space="PSUM") as psp,   # st/proj shared
            tc.tile_pool(name="up", bufs=4, space="PSUM") as up,    # U pairs + phase C
            tc.tile_pool(name="et", bufs=8) as etp,
            tc.tile_pool(name="rz", bufs=4) as rzp,
            tc.tile_pool(name="ysb", bufs=4) as ysb,
        ):
            # ---- persistent SBUF (DMAs ordered by first use) ----
            wq_sb = persist.tile([P, ND, DC], F32R)
            wk_sb = persist.tile([P, ND, DC], F32R)
            wv_sb = persist.tile([P, ND, DC], F32R)
            bq_sb = persist.tile([P, 2], F32)
            bk_sb = persist.tile([P, 2], F32)
            nc.sync.dma_start(out=wk_sb, in_=wk_t.ap().rearrange("(c p) m -> p c m", p=P))
            nc.sync.dma_start(out=bk_sb, in_=bk_s.ap())
            nc.sync.dma_start(out=bq_sb, in_=bq_s.ap())
            woc_sb = persist.tile([P, 2, D], F32R)

            qt_sb = [persist.tile([P, S], st_dtype, tag=f"qt{t}", name=f"qt{t}") for t in range(2)]
            kt_sb = [persist.tile([P, S], st_dtype, tag=f"kt{t}", name=f"kt{t}") for t in range(2)]
            vaug = persist.tile([P, NJ, G * 65], FP16)
            for h in range(G):
                nc.vector.memset(vaug[:, :, h * 65 + 64 : h * 65 + 65], 1.0)
            o_sb = persist.tile([P, NIT, DC], F32R)
            ot_sb = [persist.tile([P, S], F32R, tag=f"ot{t}", name=f"ot{t}") for t in range(2)]
            ident_f = persist.tile([P, P], F32)
            make_identity(nc, ident_f)
            ident = persist.tile([P, P], F32R)
            nc.vector.tensor_copy(ident, ident_f)

            # ---- work items: column-block DMA + projections, phase-C steps ----
            slabs = {}       # (tensor_key, cb) -> list of 8 (128, IB) tiles
            emitted = set()  # work-item keys already emitted

            HW_ = IB // 2

            def dma_slab(key, xt, cb):
                tiles = []
                for dc in range(ND):
                    t = xs.tile([P, IB], F32R, tag="x", name="x")
                    nc.sync.dma_start(
                        out=t, in_=xt.ap()[dc * P : (dc + 1) * P, cb * IB : (cb + 1) * IB]
                    )
                    tiles.append(t)
                slabs[(key, cb)] = tiles
                emitted.add(("dma", key, cb))

            def proj_qk(key, cb, mt, half, w_sb, b_sb, out_tiles):
                # half-width (256-col) groups keep the PSUM slot held for
                # less than one exp tick, so background projection work
                # never stalls the attention pipeline.
                ps = psp.tile([P, HW_], F32, tag="ps", name="ps")
                x_tiles = slabs[(key, cb)]
                for dc in range(ND):
                    nc.tensor.matmul(
                        ps,
                        w_sb[:, dc, mt * P : (mt + 1) * P],
                        x_tiles[dc][:, half * HW_ : (half + 1) * HW_],
                        start=(dc == 0),
                        stop=(dc == ND - 1),
                    )
                nc.vector.tensor_scalar(
                    out_tiles[mt][:, cb * IB + half * HW_ : cb * IB + (half + 1) * HW_],
                    ps,
                    b_sb[:, mt : mt + 1],
                    None,
                    op0=ADD,
                )
                emitted.add((key, cb, mt, half))

            def proj_v(cb, jq):
                # jq: 0..3 within the column block; global j chunk jt
                jt = cb * 4 + jq
                ps = psp.tile([P, DC], F32, tag="ps", name="ps")
                x_tiles = slabs[("v", cb)]
                for dc in range(ND):
                    nc.tensor.matmul(
                        ps,
                        x_tiles[dc][:, jq * P : (jq + 1) * P],
                        wv_sb[:, dc, :],
                        start=(dc == 0),
                        stop=(dc == ND - 1),
                    )
                nc.vector.tensor_copy(
                    vaug[:, jt].rearrange("p (h c) -> p h c", h=G)[:, :, 0:DK],
                    ps.rearrange("p (h c) -> p h c", h=G),
                )
                emitted.add(("v", cb, jq))

            def oproj(it):
                ys = ysb.tile([P, D], F32, tag="ysb", name="ysb")
                for nh in range(2):
                    yp = up.tile([P, IB], F32, tag="u", name="yp")
                    for mt in range(2):
                        nc.tensor.matmul(
                            yp,
                            ot_sb[mt][:, it * P : (it + 1) * P],
                            woc_sb[:, mt, nh * IB : (nh + 1) * IB],
                            start=(mt == 0),
                            stop=(mt == 1),
                        )
                    nc.vector.tensor_copy(ys[:, nh * IB : (nh + 1) * IB], yp)
                nc.sync.dma_start(out=y.ap()[it * P : (it + 1) * P, :], in_=ys)

            def phase_c(ib):
                items = [
                    lambda mt=mt, pair=pair: transp2(ib, mt, pair)
                    for mt in range(2)
                    for pair in range(2)
                ]
                items += [lambda it=it: oproj(it) for it in range(ib * 4, ib * 4 + 4)]
                return items

            def phase_c_late(ib):
                # hp0 transposes were already queued during the last sweep;
                # start each it-pair's output projection right after its own
                # mt1 transpose so the tail chain is as short as possible
                items = []
                for pair in range(2):
                    items.append(lambda pair=pair: transp2(ib, 1, pair))
                    for it in range(ib * 4 + 2 * pair, ib * 4 + 2 * pair + 2):
                        items.append(lambda it=it: oproj(it))
                return items

            def transp2(ib, mt, pair):
                for it in range(ib * 4 + 2 * pair, ib * 4 + 2 * pair + 2):
                    tp = up.tile([P, P], F32R, tag="u", name="tp")
                    nc.tensor.transpose(
                        tp, o_sb[:, it, mt * P : (mt + 1) * P], ident
                    )
                    nc.vector.tensor_copy(
                        ot_sb[mt][:, it * P : (it + 1) * P], tp
                    )

            work = deque()    # psum-holding items: paced to alternate ticks
            light = deque()   # DMA/staging items: drained every tick

            def drain(tick):
                if light:
                    light.popleft()()
                if work and (tick % 2 == 0 or not light):
                    work.popleft()()

            def drain_until(key):
                while key not in emitted:
                    assert work or light, f"work exhausted before {key}"
                    if light:
                        light.popleft()()
                    elif work:
                        work.popleft()()

            # column block 0 emitted up front (the pipeline fill); cb0's
            # V-projection drains inside the first J sweep instead of
            # stalling the PE stream ahead of the first score matmul.
            # column block 0 emitted up front (the pipeline fill); cb0's
            # V-projection and the hp1 (mt=1) projections drain inside the
            # first J sweep instead of stalling ahead of the first score MM.
            dma_slab("k", xk_t, 0)
            proj_qk("k", 0, 0, 0, wk_sb, bk_sb, kt_sb)
            proj_qk("k", 0, 0, 1, wk_sb, bk_sb, kt_sb)
            nc.sync.dma_start(out=wq_sb, in_=wq_t.ap().rearrange("(c p) m -> p c m", p=P))
            dma_slab("q", xq_t, 0)
            proj_qk("q", 0, 0, 0, wq_sb, bq_sb, qt_sb)
            proj_qk("q", 0, 0, 1, wq_sb, bq_sb, qt_sb)
            nc.sync.dma_start(out=wv_sb, in_=wv_t.ap().rearrange("(c p) m -> p c m", p=P))
            dma_slab("v", xv_t, 0)

            def load_woc():
                nc.sync.dma_start(
                    out=woc_sb, in_=wo_t.ap().rearrange("(t p) n -> p t n", p=P)
                )

            work.append(lambda: proj_qk("k", 0, 1, 0, wk_sb, bk_sb, kt_sb))
            work.append(lambda: proj_qk("k", 0, 1, 1, wk_sb, bk_sb, kt_sb))
            work.append(lambda: proj_qk("q", 0, 1, 0, wq_sb, bq_sb, qt_sb))
            work.append(lambda: proj_qk("q", 0, 1, 1, wq_sb, bq_sb, qt_sb))
            for jq in range(4):
                work.append(lambda jq=jq: proj_v(0, jq))
            for cb in range(1, NIB):
                light.append(lambda cb=cb: dma_slab("k", xk_t, cb))
                for half in range(2):
                    work.append(lambda cb=cb, half=half: proj_qk("k", cb, 0, half, wk_sb, bk_sb, kt_sb))
                    work.append(lambda cb=cb, half=half: proj_qk("k", cb, 1, half, wk_sb, bk_sb, kt_sb))
                light.append(lambda cb=cb: dma_slab("v", xv_t, cb))
                for jq in range(4):
                    work.append(lambda cb=cb, jq=jq: proj_v(cb, jq))
            light.append(load_woc)
            for cb in range(1, NIB):
                light.append(lambda cb=cb: dma_slab("q", xq_t, cb))
                for half in range(2):
                    work.append(lambda cb=cb, half=half: proj_qk("q", cb, 0, half, wq_sb, bq_sb, qt_sb))
                    work.append(lambda cb=cb, half=half: proj_qk("q", cb, 1, half, wq_sb, bq_sb, qt_sb))

            # ---- attention: flat software pipeline over (ib, hp, J) ticks ----
            # scores+exp for tick t are emitted one tick ahead of tick t-1's
            # P@V matmuls, so the exp stream never waits on a pair boundary.
            seq = [(0, hp, jblk * 4 + J) for jblk in range(4) for hp in range(2) for J in range(4)]
            seq += [(ib, hp, J) for ib in range(1, NIB) for hp in range(2) for J in range(NJ)]
            u_tiles = {}   # (ib, hp) -> [uA, uB]
            et_tiles = {}  # tick index -> et tile

            def emit_st_exp(idx):
                ib, hp, J = seq[idx]
                if J == 0:
                    drain_until(("q", ib, hp, 0))
                    drain_until(("q", ib, hp, 1))
                if ib == 0:
                    drain_until(("k", J // 4, hp, 0))
                    drain_until(("k", J // 4, hp, 1))
                st = psp.tile([P, 2 * IB], F32, tag="ps", name="st")
                for hx in range(2):
                    nc.tensor.matmul(
                        st[:, hx * IB : (hx + 1) * IB],
                        kt_sb[hp][hx * DK : (hx + 1) * DK, J * P : (J + 1) * P],
                        qt_sb[hp][hx * DK : (hx + 1) * DK, ib * IB : (ib + 1) * IB],
                        start=True,
                        stop=True,
                        tile_position=(hx * DK, 0),
                    )
                et = etp.tile([P, 2 * IB], FP16, tag="et", name="et")
                nc.scalar.activation(et, st, EXP, scale=float(SCALE))
                et_tiles[idx] = et
                if dbg and idx == 0:
                    nc.sync.dma_start(out=d_et.ap(), in_=et)

            def emit_pv(idx):
                ib, hp, J = seq[idx]
                if J == 0:
                    u_tiles[(ib, hp)] = [
                        up.tile([P, 4 * 65], F32, tag="u", name="u") for _ in range(2)
                    ]
                if ib == 0 and hp == 0:
                    drain_until(("v", J // 4, J % 4))
                et = et_tiles.pop(idx)
                u_hx = u_tiles[(ib, hp)]
                for hx in range(2):
                    h = 2 * hp + hx
                    for it in range(4):
                        # start clears has_written for the WHOLE bank: only the
                        # first matmul into this U bank may set it.
                        nc.tensor.matmul(
                            u_hx[hx][:, it * 65 : (it + 1) * 65],
                            et[:, hx * IB + it * P : hx * IB + (it + 1) * P],
                            vaug[:, J, h * 65 : (h + 1) * 65],
                            start=(J == 0 and it == 0),
                            stop=(J == NJ - 1 and it == 3),
                            skip_group_check=True,
                        )
                if J == NJ - 1:
                    finish_pair(ib, hp)

            def finish_pair(ib, hp):
                u_hx = u_tiles.pop((ib, hp))
                if dbg and ib == 0 and hp == 0:
                    for hx in range(2):
                        du_sb = ysb.tile([P, 4 * 65], F32, tag="du", name="du")
                        nc.vector.tensor_copy(du_sb, u_hx[hx])
                        nc.sync.dma_start(out=d_u.ap()[hx], in_=du_sb)
                for hx in range(2):
                    h = 2 * hp + hx
                    uv = u_hx[hx].rearrange("p (r c) -> p r c", c=65)
                    rz = rzp.tile([P, 4, 1], F32, tag="rz", name="rz")
                    nc.vector.reciprocal(rz, uv[:, :, 64:65])
                    for it in range(4):
                        nc.vector.tensor_scalar(
                            o_sb[:, ib * 4 + it, h * DK : (h + 1) * DK],
                            uv[:, it, 0:DK],
                            rz[:, it],
                            None,
                            op0=MULT,
                        )
                if hp == 0 and ib == NIB - 1:
                    # last i-block: transpose the hp0 heads during the final
                    # sweep so less phase-C work serializes after the last exp
                    work.extend(
                        [lambda pair=pair: transp2(NIB - 1, 0, pair) for pair in range(2)]
                    )
                elif hp == 1:
                    items = phase_c(ib) if ib < NIB - 1 else phase_c_late(ib)
                    work.extend(items)

            for idx in range(len(seq) + 1):
                if idx < len(seq):
                    emit_st_exp(idx)
                if idx >= 1:
                    emit_pv(idx - 1)
                drain(idx)

            while work or light:
                (light or work).popleft()()

            if dbg:
                for t in range(2):
                    qsrc = qt_sb[t].bitcast(qk_dt) if st_dtype == F32R else qt_sb[t]
                    ksrc = kt_sb[t].bitcast(qk_dt) if st_dtype == F32R else kt_sb[t]
                    nc.sync.dma_start(out=d_qt.ap()[t], in_=qsrc)
                    nc.sync.dma_start(out=d_kt.ap()[t], in_=ksrc)
                nc.sync.dma_start(out=d_va.ap(), in_=vaug)
                nc.sync.dma_start(out=d_o.ap(), in_=o_sb.bitcast(F32))

    nc.compile()
    return nc


def _get_nc():
    global _NC_CACHE
    if _NC_CACHE is None:
        _NC_CACHE = _build()
    return _NC_CACHE


def _in_maps(query, key, value, wq, wk, wv, wo, bq, bk):
    maps = []
    for c in range(8):
        b, g = divmod(c, 4)
        sl = slice(g * DC, (g + 1) * DC)
        maps.append(
            {
                "xq_t": np.ascontiguousarray(query[:, b, :].T),
                "xk_t": np.ascontiguousarray(key[:, b, :].T),
                "xv_t": np.ascontiguousarray(value[:, b, :].T),
                "wq_t": np.ascontiguousarray(wq[sl, :].T),
                "wk_t": np.ascontiguousarray(wk[sl, :].T),
                "wv_t": np.ascontiguousarray(wv[sl, :].T),
                "wo_t": np.ascontiguousarray(wo[:, sl].T),
                "bq_s": np.ascontiguousarray(bq[sl].reshape(2, P).T),
                "bk_s": np.ascontiguousarray(bk[sl].reshape(2, P).T),
            }
        )
    return maps


def kernel(
    query, key, value, wq, bq, wk, bk, wv, bv, wo, bo, **_kw
) -> np.ndarray:
    query = np.asarray(query, np.float32)
    key = np.asarray(key, np.float32)
    value = np.asarray(value, np.float32)
    wq = np.asarray(wq, np.float32)
    wk = np.asarray(wk, np.float32)
    wv = np.asarray(wv, np.float32)
    wo = np.asarray(wo, np.float32)
    bq = np.asarray(bq, np.float32)
    bk = np.asarray(bk, np.float32)
    bv = np.asarray(bv, np.float32)
    bo = np.asarray(bo, np.float32)

    nc = _get_nc()
    res = run_bass_kernel_spmd(
        nc, _in_maps(query, key, value, wq, wk, wv, wo, bq, bk),
        core_ids=list(range(8)),
    )

    out = np.zeros((S, B, D), np.float32)
    for c in range(8):
        out[:, c // 4, :] += res.results[c]["y"]
    out += bo + wo @ bv
    return out
